# revision 1
# baseline (speedup 1.0000x reference)
"""Trainium2 Bass kernel for nn_Block_56427280335230 (dense transformer block).

Reference semantics (B=2, L=2048, H=16, D=64, HID=1024):
    h = LayerNorm(x) * ln_w + ln_b
    h[..., :128] = cumlogsumexp(h[..., :128] * 5, axis=seq) / 5
    qkvp = h @ w_in.T ; split q,k,v,p
    q,k = rope(q), rope(k)
    o = softmax(q k^T / 8 + causal) v
    out = concat([o, gelu(p)]) @ w_out.T + b_out

Sharding: DP2 x TP4 over 8 NeuronCores. Cores 0-3 take batch 0, cores 4-7
batch 1. Within a group of 4, heads (4 per core) and the qkvp/vp weight
columns are sharded. Each core computes a full partial out^T [1024, 2048]
over its vp shard; a ReduceScatter over the 4-core group leaves each core
with a disjoint 256-channel slice of the summed output. The host
concatenates the 8 disjoint shards (pure gather, no host reduction).

On-device dataflow is feature-major (channels on partitions, tokens on the
free axis) end to end, so no activation transposes are needed. LayerNorm is
folded into the QKVP matmul via two augmented contraction rows (-mu and
sqrt(var+eps)) with host-augmented weights; the per-token rstd scale is
applied on the PSUM->SBUF pass. The soft-prefix-max uses the DVE prefix-scan
(exp -> cumsum -> ln). Attention computes S^T blocks (keys on partitions) so
exp(S^T) @ V needs no transposes; the softmax denominator rides as a
ones-column in the AV matmul. All matmuls are bf16 with fp32 accumulation.
"""
import numpy as np
import ml_dtypes
from contextlib import ExitStack

from concourse import bass, mybir, tile, bacc
from concourse.masks import make_identity

F32 = mybir.dt.float32
BF16 = mybir.dt.bfloat16

B, L, H, D = 2, 2048, 16, 64
HID = H * D                  # 1024
ACC = HID // 8               # 128 scan channels
N_CORES = 8
TP = 4                       # tensor-parallel group size
HPC = H // TP                # heads per core = 4
CH = 512                     # tokens per chunk
NCH = L // CH                # 4 chunks
KB = 128                     # key block
NKB = L // KB                # 16 key blocks
KQ = 8                       # qkvp contraction tiles: 7 centered-x + 1 part
MQK, MV, MP = HPC, HPC // 2, 8
MTOT = MQK + MV + MP         # 14 m-tiles of the qkvp output (1792 rows)
MO = 8                       # out-proj m-tiles (1024 out channels)
KO = 10                      # out-proj contraction tiles (1280 vp shard)
VP_SH = KO * 128             # 1280
RG = [[0, 1, 2, 3], [4, 5, 6, 7]]

AF = mybir.ActivationFunctionType
OP = mybir.AluOpType


def build_nc(sim_safe=False, debug_partial=False, skip_collective=False):
    nc = bacc.Bacc("TRN2", target_bir_lowering=False, debug=False,
                   num_devices=N_CORES)
    ap = {}
    ins_spec = [
        ("xt", [HID, L], BF16),
        ("wq", [KQ * 128, MTOT * 128], BF16),
        ("wo", [VP_SH, MO * 128], BF16),
        ("cos2", [128, L], BF16),
        ("sin2", [128, L], BF16),
        ("tri", [128, 128], BF16),
        ("lnw0", [128, 1], F32),
        ("lnb0", [128, 1], F32),
        ("b4", [128, MO], F32),
    ]
    for name, shape, dt in ins_spec:
        ap[name] = nc.dram_tensor(name, shape, dt, kind="ExternalInput").ap()
    out_sh = nc.dram_tensor("out_sh", [HID // TP, L], F32, kind="ExternalOutput").ap()
    if debug_partial:
        partial_dbg = nc.dram_tensor("partial", [HID, L], F32, kind="ExternalOutput").ap()

    with tile.TileContext(nc) as tc, ExitStack() as ctx:
        ctx.enter_context(nc.allow_low_precision(
            reason="bf16 compute pipeline by design; fp32 accumulation in PSUM"))
        wp = ctx.enter_context(tc.tile_pool(name="wp", bufs=1))
        xp = ctx.enter_context(tc.tile_pool(name="xp", bufs=2))
        bp = ctx.enter_context(tc.tile_pool(name="bp", bufs=3))
        tp_ = ctx.enter_context(tc.tile_pool(name="tp", bufs=2))
        rp = ctx.enter_context(tc.tile_pool(name="rp", bufs=1))
        pep = ctx.enter_context(tc.tile_pool(name="pep", bufs=4))
        stp = ctx.enter_context(tc.tile_pool(name="stp", bufs=3))
        psmm = ctx.enter_context(tc.tile_pool(name="psmm", bufs=2, space="PSUM"))
        psst = ctx.enter_context(tc.tile_pool(name="psst", bufs=1, space="PSUM"))
        pspt = ctx.enter_context(tc.tile_pool(name="pspt", bufs=2, space="PSUM"))
        psot = ctx.enter_context(tc.tile_pool(name="psot", bufs=2, space="PSUM"))
        dram = ctx.enter_context(tc.tile_pool(name="dram", bufs=1, space="DRAM"))

        # ---- prefetch x chunk 0 before the weight bulk so stats matmuls
        # and the first qkvp m-tile start as early as possible ----
        xt3 = ap["xt"].rearrange("(a p) t -> p a t", p=128)   # [128, 8, L]
        xc0 = xp.tile([128, 8 * CH], BF16, tag="xc", name="xc0")
        nc.gpsimd.dma_start(out=xc0[:].rearrange("p (a t) -> p a t", a=8),
                          in_=xt3[:, :, 0:CH])

        # ---- resident weights / constants ----
        # qk+v columns first: the first chunk's qk/v m-tiles can start while
        # the p columns and wo are still in flight.
        QKV_COLS = (MQK + MV) * 128  # 768
        wq3 = ap["wq"].rearrange("(a p) m -> p a m", p=128)    # [128, 9, 1792]
        wq_sb = wp.tile([128, KQ * MTOT * 128], BF16)   # [128, 9*1792]
        wq_sb3 = wq_sb[:].rearrange("p (a m) -> p a m", a=KQ)
        nc.gpsimd.dma_start(out=wq_sb3[:, :, 0:QKV_COLS], in_=wq3[:, :, 0:QKV_COLS])
        nc.gpsimd.dma_start(out=wq_sb3[:, :, QKV_COLS:], in_=wq3[:, :, QKV_COLS:])
        cos_sb = wp.tile([128, L], BF16)
        sin_sb = wp.tile([128, L], BF16)
        tri_sb = wp.tile([128, 128], BF16)
        nc.gpsimd.dma_start(out=cos_sb, in_=ap["cos2"])
        nc.gpsimd.dma_start(out=sin_sb, in_=ap["sin2"])
        nc.sync.dma_start(out=tri_sb, in_=ap["tri"])
        wo_sb = wp.tile([128, KO * MO * 128], BF16)     # [128, 10*1024]

        def load_wo():
            nc.gpsimd.dma_start(
                out=wo_sb[:].rearrange("p (a m) -> p a m", a=KO),
                in_=ap["wo"].rearrange("(a p) m -> p a m", p=128))
        lnw0 = wp.tile([128, 1], F32)
        lnb0 = wp.tile([128, 1], F32)
        b4_sb = wp.tile([128, MO], F32)
        nc.sync.dma_start(out=lnw0, in_=ap["lnw0"])
        nc.sync.dma_start(out=lnb0, in_=ap["lnb0"])
        nc.sync.dma_start(out=b4_sb, in_=ap["b4"])
        ones_sb = wp.tile([128, 1], BF16)
        nc.vector.memset(ones_sb, 1.0 / HID)
        ident = wp.tile([128, 128], BF16)
        make_identity(nc, ident)
        eps_sb = wp.tile([1, 1], F32)
        nc.vector.memset(eps_sb, 1e-5)
        carry = wp.tile([128, 1], F32)

        qk_t = [wp.tile([128, L], BF16, tag=f"qk{i}", name=f"qk{i}") for i in range(4)]  # qq01,kk01,qq23,kk23
        vaug = [wp.tile([128, NKB, 65], BF16, tag=f"v{h}", name=f"v{h}") for h in range(HPC)]
        for h in range(HPC):
            nc.vector.memset(vaug[h][:, :, 64:65], 1.0)
        pp = ctx.enter_context(tc.tile_pool(name="pp", bufs=2))

        bounce_in = [dram.tile([HID, CH], F32, name=f"rsin{c}") for c in range(NCH)]
        bounce_out = [dram.tile([HID // TP, CH], F32, name=f"rsout{c}") for c in range(NCH)]

        def pre_phase(c, xc):
            """Stats + soft-prefix scan + aug/partT rhs tiles for chunk c.

            Emitted one chunk AHEAD of its qkvp matmuls (between chunk c-1's
            attention and out-proj) so the stats->DVE->broadcast->aug chain
            is off the PE critical path at chunk boundaries.
            """
            # ---- stats: mean and mean-square via ones-matmul ----
            mu_ps = psst.tile([1, CH], F32, tag="st0", name=f"mu_ps{c}")
            sq_ps = psst.tile([1, CH], F32, tag="st1", name=f"sq_ps{c}")
            for kt in range(8):
                nc.tensor.matmul(mu_ps, ones_sb, xc[:, kt * CH:(kt + 1) * CH],
                                 start=(kt == 0), stop=(kt == 7))
            for kt in range(8):
                sq = xp.tile([128, CH], BF16, tag="sq", name=f"sq{c}_{kt}")
                nc.scalar.activation(out=sq, in_=xc[:, kt * CH:(kt + 1) * CH],
                                     func=AF.Square)
                nc.tensor.matmul(sq_ps, ones_sb, sq,
                                 start=(kt == 0), stop=(kt == 7))
            mu_row = rp.tile([1, CH], F32, tag="mu_row", bufs=2)
            nc.vector.tensor_copy(out=mu_row, in_=mu_ps)
            mu_bf = rp.tile([1, CH], BF16, tag="mu_bf", bufs=2)
            nc.vector.tensor_copy(out=mu_bf, in_=mu_ps)
            var_row = rp.tile([1, CH], F32, tag="var", bufs=2)
            nc.vector.scalar_tensor_tensor(out=var_row, in0=mu_row, scalar=-1.0,
                                           in1=mu_ps, op0=OP.mult, op1=OP.mult)
            nc.vector.tensor_add(out=var_row, in0=var_row, in1=sq_ps)
            sqv_row = rp.tile([1, CH], F32, tag="sqv", bufs=2)
            nc.scalar.activation(out=sqv_row, in_=var_row, func=AF.Sqrt,
                                 bias=eps_sb, scale=1.0)
            rstd_row = rp.tile([1, CH], F32, tag="rstd", bufs=2)
            nc.vector.reciprocal(out=rstd_row, in_=sqv_row)
            mu_b = bp.tile([128, CH], BF16, tag="mu_b", name=f"mu_b{c}")
            rstd_b = bp.tile([128, CH], F32, tag="rstd_b", name=f"rstd_b{c}")
            sqv_b = bp.tile([128, CH], F32, tag="sqv_b", name=f"sqv_b{c}")
            nc.gpsimd.partition_broadcast(mu_b, mu_bf)
            nc.gpsimd.partition_broadcast(rstd_b, rstd_row)
            nc.gpsimd.partition_broadcast(sqv_b, sqv_row)

            # ---- center x in place (channels 128..1023): x <- x - mu.
            # Replaces the aug contraction tile; runs off the PE critical
            # path thanks to the one-chunk-ahead pre-phase pipelining.
            for kt in range(1, 8):
                nc.vector.tensor_tensor(
                    out=xc[:, kt * CH:(kt + 1) * CH],
                    in0=xc[:, kt * CH:(kt + 1) * CH], in1=mu_b,
                    op=OP.subtract)

            # ---- soft prefix max on channels 0-127 ----
            h0 = tp_.tile([128, CH], F32, tag="h0", name=f"h0_{c}")
            nc.vector.tensor_tensor(out=h0, in0=xc[:, 0:CH], in1=mu_b,
                                    op=OP.subtract)
            nc.vector.tensor_mul(out=h0, in0=h0, in1=rstd_b)
            nc.vector.tensor_scalar(out=h0, in0=h0, scalar1=lnw0, scalar2=lnb0,
                                    op0=OP.mult, op1=OP.add)
            e0 = tp_.tile([128, CH], BF16, tag="e0", name=f"e0_{c}")
            nc.scalar.activation(out=e0, in_=h0, func=AF.Exp, scale=5.0)
            c0 = tp_.tile([128, CH], F32, tag="c0", name=f"c0_{c}")
            nc.vector.tensor_tensor_scan(
                out=c0, data0=e0, data1=e0,
                initial=(0.0 if c == 0 else carry[:, 0:1]),
                op0=OP.add, op1=OP.bypass)
            nc.vector.tensor_copy(out=carry, in_=c0[:, CH - 1:CH])
            lnc = tp_.tile([128, CH], BF16, tag="lnc", name=f"lnc{c}")
            nc.scalar.activation(out=lnc, in_=c0, func=AF.Ln)
            partT = tp_.tile([128, CH], BF16, tag="partT", name=f"partT{c}")
            nc.vector.tensor_mul(out=partT, in0=lnc, in1=sqv_b)
            return dict(partT=partT, rstd_b=rstd_b)

        xcs = {0: xc0}
        pres = {0: pre_phase(0, xc0)}
        for c in range(NCH):
            t0, t1 = c * CH, (c + 1) * CH
            xc = xcs[c]
            partT, rstd_b = pres[c]["partT"], pres[c]["rstd_b"]
            p_t = [pp.tile([128, CH], BF16, tag=f"p{i}", name=f"p{i}_{c}")
                   for i in range(MP)]
            cat01 = pp.tile([128, CH], BF16, tag="cat01", name=f"cat01_{c}")
            cat23 = pp.tile([128, CH], BF16, tag="cat23", name=f"cat23_{c}")

            # ---- qkvp projection: 14 m-tiles x 9 k-tiles ----
            rhs_tiles = [xc[:, kt * CH:(kt + 1) * CH] for kt in range(1, 8)]
            rhs_tiles += [partT]
            for mt in range(MTOT):
                mm = psmm.tile([128, CH], F32, tag="mm")
                for kt in range(KQ):
                    nc.tensor.matmul(
                        mm,
                        wq_sb[:, kt * 1792 + mt * 128: kt * 1792 + (mt + 1) * 128],
                        rhs_tiles[kt],
                        start=(kt == 0), stop=(kt == KQ - 1))
                if mt < MQK:
                    qks = tp_.tile([128, CH], BF16, tag="qks")
                    nc.vector.tensor_mul(out=qks, in0=mm, in1=rstd_b)
                    rot = tp_.tile([128, CH], BF16, tag="rot")
                    nc.vector.tensor_copy(out=rot[0:32], in_=qks[32:64])
                    nc.vector.tensor_copy(out=rot[32:64], in_=qks[0:32])
                    nc.vector.tensor_copy(out=rot[64:96], in_=qks[96:128])
                    nc.vector.tensor_copy(out=rot[96:128], in_=qks[64:96])
                    qc = tp_.tile([128, CH], BF16, tag="qc")
                    nc.vector.tensor_mul(out=qc, in0=qks, in1=cos_sb[:, t0:t1])
                    nc.vector.tensor_mul(out=rot, in0=rot, in1=sin_sb[:, t0:t1])
                    nc.vector.tensor_add(out=qk_t[mt][:, t0:t1], in0=qc, in1=rot)
                elif mt < MQK + MV:
                    vi = mt - MQK
                    v_sb = tp_.tile([128, CH], BF16, tag="v_sb")
                    nc.vector.tensor_mul(out=v_sb, in0=mm, in1=rstd_b)
                    for half in range(2):
                        h = 2 * vi + half
                        for blk in range(CH // KB):
                            jb = (CH // KB) * c + blk
                            tr = pspt.tile([128, 64], BF16, tag="pt")
                            nc.tensor.transpose(
                                tr, v_sb[64 * half:64 * half + 64,
                                         blk * KB:(blk + 1) * KB],
                                ident[64 * half:64 * half + 64,
                                      64 * half:64 * half + 64])
                            nc.vector.tensor_copy(out=vaug[h][:, jb, 0:64], in_=tr)
                else:
                    pi = mt - MQK - MV
                    pf = tp_.tile([128, CH], BF16, tag="pf")
                    nc.vector.tensor_mul(out=pf, in0=mm, in1=rstd_b)
                    if sim_safe:
                        sg = tp_.tile([128, CH], BF16, tag="sg")
                        nc.scalar.activation(out=sg, in_=pf, func=AF.Sigmoid,
                                             scale=1.702)
                        nc.vector.tensor_mul(out=p_t[pi], in0=pf, in1=sg)
                    else:
                        nc.scalar.activation(out=p_t[pi], in_=pf, func=AF.Gelu)

            if c == 0:
                load_wo()

            # ---- attention for this q-chunk, two heads at a time ----
            # Heads of a pair use PE row groups 0-63 / 64-127, so their PT
            # matmuls run concurrently in the array.
            nblk = (CH // KB) * (c + 1)
            for pair in range(HPC // 2):
                qq = qk_t[2 * pair]
                kk = qk_t[2 * pair + 1]
                ots = [psot.tile([65, CH], F32, tag="ot", name=f"ot{c}_{pair}_{i}")
                       for i in range(2)]
                for j in range(nblk):
                    dm = j - (CH // KB) * c
                    qlo = KB * dm if dm >= 0 else 0
                    pts, pes = [], []
                    for i in range(2):
                        sl = slice(64 * i, 64 * i + 64)
                        pt = pspt.tile([128, CH], F32, tag="pt",
                                       name=f"pt{c}_{pair}_{j}_{i}")
                        nc.tensor.matmul(
                            pt[:, qlo:CH],
                            kk[sl, j * KB:(j + 1) * KB],
                            qq[sl, t0 + qlo:t1],
                            start=True, stop=True)
                        pts.append(pt)
                    for i in range(2):
                        pe = pep.tile([128, CH], BF16, tag="pe",
                                      name=f"pe{c}_{pair}_{j}_{i}")
                        nc.scalar.activation(out=pe[:, qlo:CH],
                                             in_=pts[i][:, qlo:CH], func=AF.Exp)
                        if dm >= 0:
                            nc.vector.tensor_mul(out=pe[:, qlo:qlo + KB],
                                                 in0=pe[:, qlo:qlo + KB],
                                                 in1=tri_sb)
                        pes.append(pe)
                    for i in range(2):
                        h = 2 * pair + i
                        nc.tensor.matmul(
                            ots[i][:, qlo:CH], vaug[h][:, j, :], pes[i][:, qlo:CH],
                            start=(j == 0), stop=(j == nblk - 1),
                            skip_group_check=True)
                for i in range(2):
                    h = 2 * pair + i
                    ot = ots[i]
                    den = rp.tile([1, CH], BF16, tag="den", bufs=2)
                    nc.vector.reciprocal(out=den, in_=ot[64:65, :])
                    den_b = bp.tile([64, CH], BF16, tag="den_b")
                    nc.gpsimd.partition_broadcast(den_b, den)
                    dest = cat01 if h < 2 else cat23
                    r0 = 64 * (h % 2)
                    nc.vector.tensor_mul(out=dest[r0:r0 + 64, :],
                                         in0=ot[0:64, :], in1=den_b)

            # ---- pipeline chunk c+1's x load and pre-phase here, so its
            # stats/scan/broadcast chain overlaps this chunk's out-proj ----
            if c + 1 < NCH:
                xn = xp.tile([128, 8 * CH], BF16, tag="xc", name=f"xc{c + 1}")
                nc.gpsimd.dma_start(out=xn[:].rearrange("p (a t) -> p a t", a=8),
                                    in_=xt3[:, :, (c + 1) * CH:(c + 2) * CH])
                xcs[c + 1] = xn
                pres[c + 1] = pre_phase(c + 1, xn)

            # ---- out-proj for this chunk: 8 m-tiles x 10 k-tiles ----
            orhs = [cat01, cat23] + [p_t[i] for i in range(MP)]
            kt_order = list(range(2, KO)) + [0, 1]   # p first, cat last
            bn3 = bounce_in[c][:].rearrange("(a p) t -> p a t", p=128)
            for mg in range(MO // 2):
                st = stp.tile([128, 2 * CH], F32, tag="st")
                for mi in range(2):
                    mt = 2 * mg + mi
                    mm = psmm.tile([128, CH], F32, tag="mm")
                    for ki, kt in enumerate(kt_order):
                        nc.tensor.matmul(
                            mm,
                            wo_sb[:, kt * 1024 + mt * 128: kt * 1024 + (mt + 1) * 128],
                            orhs[kt],
                            start=(ki == 0), stop=(ki == KO - 1))
                    nc.scalar.activation(out=st[:, mi * CH:(mi + 1) * CH],
                                         in_=mm, func=AF.Identity,
                                         bias=b4_sb[:, mt:mt + 1], scale=1.0)
                nc.gpsimd.dma_start(
                    out=bn3[:, 2 * mg:2 * mg + 2, :],
                    in_=st[:].rearrange("p (a t) -> p a t", a=2))

            # ---- chunked reduce-scatter: overlap comm with later chunks ----
            if not skip_collective:
                nc.gpsimd.collective_compute(
                    "ReduceScatter", OP.add,
                    ins=[bounce_in[c][:].opt()],
                    outs=[bounce_out[c][:].opt()],
                    replica_groups=RG)
                nc.gpsimd.dma_start(out=out_sh[:, t0:t1],
                                    in_=bounce_out[c][:])

        if debug_partial:
            for c in range(NCH):
                nc.sync.dma_start(out=partial_dbg[:, c * CH:(c + 1) * CH],
                                  in_=bounce_in[c][:])
    nc.compile()
    return nc


# ---------------- host-side sharding ----------------

def _rope_tables():
    inv = 1.0 / (10000.0 ** (np.arange(0, D, 2, dtype=np.float64) / D))
    t = np.arange(L, dtype=np.float64)
    f = t[:, None] * inv[None, :]                 # [L, 32]
    emb = np.concatenate([f, f], axis=1)          # [L, 64]
    cos64 = np.cos(emb).T                         # [64, L]
    sin64 = np.sin(emb).T
    s32 = sin64[0:32]
    sin_signed = np.concatenate([-s32, s32], axis=0)   # [64, L]
    cos2 = np.concatenate([cos64, cos64], axis=0)
    sin2 = np.concatenate([sin_signed, sin_signed], axis=0)
    bf = ml_dtypes.bfloat16
    return cos2.astype(bf), sin2.astype(bf)


def prep_inputs(x, ln_w, ln_b, w_in, w_out, b_out):
    x = np.asarray(x, np.float32)
    ln_w = np.asarray(ln_w, np.float32)
    ln_b = np.asarray(ln_b, np.float32)
    w_in = np.asarray(w_in, np.float32)
    w_out = np.asarray(w_out, np.float32)
    b_out = np.asarray(b_out, np.float32)

    cos2, sin2 = _rope_tables()
    tri = (np.arange(128)[None, :] >= np.arange(128)[:, None]).astype(ml_dtypes.bfloat16)
    lnw0 = ln_w[0:128, None].astype(np.float32)
    lnb0 = ln_b[0:128, None].astype(np.float32)
    b4 = (b_out / TP).reshape(MO, 128).T.astype(np.float32).copy()

    xt_b = [np.ascontiguousarray(x[b].T).astype(ml_dtypes.bfloat16) for b in range(B)]

    in_maps = []
    for c in range(N_CORES):
        b, tpi = divmod(c, TP)
        heads = [HPC * tpi + j for j in range(HPC)]
        # qkvp shard row order: q0,q1, k0,k1, q2,q3, k2,k3, v0..v3, p(1024)
        rows = []
        for pair in range(HPC // 2):
            h0, h1 = heads[2 * pair], heads[2 * pair + 1]
            rows += list(range(64 * h0, 64 * h0 + 64))
            rows += list(range(64 * h1, 64 * h1 + 64))
            rows += list(range(HID + 64 * h0, HID + 64 * h0 + 64))
            rows += list(range(HID + 64 * h1, HID + 64 * h1 + 64))
        for h in heads:
            rows += list(range(2 * HID + 64 * h, 2 * HID + 64 * h + 64))  # v_h
        rows += list(range(3 * HID + 1024 * tpi, 3 * HID + 1024 * (tpi + 1)))
        rows = np.array(rows)
        W_sh = w_in[rows, :]                                    # [1792, 1024]
        w_eff = W_sh * ln_w[None, :]
        c1 = W_sh[:, ACC:] @ ln_b[ACC:]
        assert np.abs(c1).max() < 1e-6, (
            "nonzero ln_b[128:] not supported by this build (c1 term dropped)")
        wq = np.zeros((KQ * 128, MTOT * 128), np.float32)
        wq[0:896] = w_eff[:, ACC:].T                            # channels 128..1023
        wq[896:1024] = 0.2 * W_sh[:, 0:ACC].T
        qs = 1.0 / float(D) ** 0.5                # fold q * D^-1/2 into weights
        wq[:, 0:128] *= qs
        wq[:, 256:384] *= qs
        # out-proj shard: columns [256*tpi:256*(tpi+1)] (o) + [1024+1024*tpi ...] (p)
        ocols = list(range(256 * tpi, 256 * (tpi + 1)))
        pcols = list(range(HID + 1024 * tpi, HID + 1024 * (tpi + 1)))
        wo = w_out[:, ocols + pcols].T                          # [1280, 1024]
        in_maps.append({
            "xt": xt_b[b],
            "wq": wq.astype(ml_dtypes.bfloat16),
            "wo": np.ascontiguousarray(wo).astype(ml_dtypes.bfloat16),
            "cos2": cos2, "sin2": sin2, "tri": tri,
            "lnw0": lnw0, "lnb0": lnb0, "b4": b4,
        })
    return in_maps


def assemble(results):
    """results: list of 8 per-core dicts with 'out_sh' [256, L] f32."""
    out = np.empty((B, L, HID), np.float32)
    for c in range(N_CORES):
        b, tpi = divmod(c, TP)
        out[b, :, 256 * tpi:256 * (tpi + 1)] = results[c]["out_sh"].T
    return out


_NC_CACHE = {}


def _get_nc():
    if "nc" not in _NC_CACHE:
        _NC_CACHE["nc"] = build_nc()
    return _NC_CACHE["nc"]


def kernel(x, ln_w, ln_b, w_in, w_out, b_out):
    from concourse.bass_utils import run_bass_kernel_spmd
    in_maps = prep_inputs(x, ln_w, ln_b, w_in, w_out, b_out)
    nc = _get_nc()
    res = run_bass_kernel_spmd(nc, in_maps, core_ids=list(range(N_CORES)))
    return assemble(res.results)



# revision 13
# speedup vs baseline: 1.0272x; 1.0272x over previous
"""Trainium2 Bass kernel for nn_Block_56427280335230 (dense transformer block).

Reference semantics (B=2, L=2048, H=16, D=64, HID=1024):
    h = LayerNorm(x) * ln_w + ln_b
    h[..., :128] = cumlogsumexp(h[..., :128] * 5, axis=seq) / 5
    qkvp = h @ w_in.T ; split q,k,v,p
    q,k = rope(q), rope(k)
    o = softmax(q k^T / 8 + causal) v
    out = concat([o, gelu(p)]) @ w_out.T + b_out

Sharding: DP2 x TP4 over 8 NeuronCores (cores 0-3 batch 0, 4-7 batch 1;
heads + qkvp/vp weight columns sharded within each group of 4; chunked
ReduceScatter leaves each core a disjoint 256-channel slice).

Dataflow is feature-major (channels on partitions, tokens free).  The
moving side of the qkvp matmul is (x - mu) * rstd (rstd = exp(-.5 ln(var
+ eps)), computed without sqrt/reciprocal so the activation table never
leaves the exp/ln set except for gelu).  q/k/v projection rows and the
attention AV matmul run as fp8e4 DoubleRow matmuls (2 k-tiles per
instruction at 0.5 cycles/row = 4x bf16 throughput); the p/gelu path and
out-proj stay bf16 since coherent fp8 error there would break the 2e-2
gate.  Weights for the fp8 side are host-scaled by 32 and descaled in the
PSUM eviction.  exp(S^T) writes fp8 directly except diagonal triangle
blocks, which stage through bf16 for the causal mask multiply.
"""
import numpy as np
import ml_dtypes
from contextlib import ExitStack

from concourse import bass, mybir, tile, bacc
from concourse.masks import make_identity

F32 = mybir.dt.float32
BF16 = mybir.dt.bfloat16
F8 = mybir.dt.float8e4
DR = mybir.MatmulPerfMode.DoubleRow

B, L, H, D = 2, 2048, 16, 64
HID = H * D                  # 1024
ACC = HID // 8               # 128 scan channels
N_CORES = 8
TP = 4                       # tensor-parallel group size
HPC = H // TP                # heads per core = 4
CH = 512                     # tokens per chunk
NCH = L // CH                # 4 chunks
KB = 128                     # key block
NKB = L // KB                # 16 key blocks
KQ = 8                       # qkvp contraction tiles (partT + 7 x tiles)
MQK, MV, MP = HPC, HPC // 2, 8
MF8 = MQK + MV               # 6 fp8 m-tiles (q,k,v)
MO = 8                       # out-proj m-tiles (1024 out channels)
KO = 10                      # out-proj contraction tiles (1280 vp shard)
VP_SH = KO * 128             # 1280
SW = 32.0                    # fp8 weight scale
RG = [[0, 1, 2, 3], [4, 5, 6, 7]]

AF = mybir.ActivationFunctionType
OP = mybir.AluOpType


def build_nc(sim_safe=False, debug_partial=False, skip_collective=False):
    nc = bacc.Bacc("TRN2", target_bir_lowering=False, debug=False,
                   num_devices=N_CORES)
    ap = {}
    ins_spec = [
        ("xt", [HID, L], BF16),
        ("wq8", [KQ * 128, MF8 * 128], F8),
        ("wqb", [KQ * 128, MP * 128], BF16),
        ("wo", [VP_SH, MO * 128], BF16),
        ("cos2", [128, L], BF16),
        ("sin2", [128, L], BF16),
        ("tri", [128, 128], BF16),
        ("lnw0", [128, 1], F32),
        ("lnb0", [128, 1], F32),
        ("b4", [128, MO], F32),
    ]
    for name, shape, dt in ins_spec:
        ap[name] = nc.dram_tensor(name, shape, dt, kind="ExternalInput").ap()
    out_sh = nc.dram_tensor("out_sh", [HID // TP, L], F32, kind="ExternalOutput").ap()
    if debug_partial:
        partial_dbg = nc.dram_tensor("partial", [HID, L], F32, kind="ExternalOutput").ap()

    with tile.TileContext(nc) as tc, ExitStack() as ctx:
        ctx.enter_context(nc.allow_low_precision(
            reason="bf16/fp8 compute pipeline by design; fp32 accumulation in PSUM"))
        wp = ctx.enter_context(tc.tile_pool(name="wp", bufs=1))
        xp = ctx.enter_context(tc.tile_pool(name="xp", bufs=2))
        hp = ctx.enter_context(tc.tile_pool(name="hp", bufs=2))
        bp = ctx.enter_context(tc.tile_pool(name="bp", bufs=3))
        tp_ = ctx.enter_context(tc.tile_pool(name="tp", bufs=2))
        rp = ctx.enter_context(tc.tile_pool(name="rp", bufs=1))
        pep = ctx.enter_context(tc.tile_pool(name="pep", bufs=4))
        stp = ctx.enter_context(tc.tile_pool(name="stp", bufs=3))
        orp = ctx.enter_context(tc.tile_pool(name="orp", bufs=2))
        psmm = ctx.enter_context(tc.tile_pool(name="psmm", bufs=2, space="PSUM"))
        psst = ctx.enter_context(tc.tile_pool(name="psst", bufs=1, space="PSUM"))
        pspt = ctx.enter_context(tc.tile_pool(name="pspt", bufs=2, space="PSUM"))
        psot = ctx.enter_context(tc.tile_pool(name="psot", bufs=2, space="PSUM"))
        dram = ctx.enter_context(tc.tile_pool(name="dram", bufs=1, space="DRAM"))

        # ---- prefetch x chunk 0 before the weight bulk ----
        xt3 = ap["xt"].rearrange("(a p) t -> p a t", p=128)   # [128, 8, L]
        xc0 = xp.tile([128, 8, CH], BF16, tag="xc", name="xc0")
        nc.gpsimd.dma_start(out=xc0, in_=xt3[:, :, 0:CH])

        # ---- resident weights / constants ----
        wq8_sb = wp.tile([128, KQ, MF8 * 128], F8)
        nc.gpsimd.dma_start(out=wq8_sb,
                            in_=ap["wq8"].rearrange("(a p) m -> p a m", p=128))
        wqb_sb = wp.tile([128, KQ, MP * 128], BF16)
        nc.gpsimd.dma_start(out=wqb_sb,
                            in_=ap["wqb"].rearrange("(a p) m -> p a m", p=128))
        cos_sb = wp.tile([128, L], BF16)
        sin_sb = wp.tile([128, L], BF16)
        tri_sb = wp.tile([128, 128], BF16)
        nc.gpsimd.dma_start(out=cos_sb, in_=ap["cos2"])
        nc.gpsimd.dma_start(out=sin_sb, in_=ap["sin2"])
        nc.sync.dma_start(out=tri_sb, in_=ap["tri"])
        wo_sb = wp.tile([128, KO, MO * 128], BF16)

        def load_wo():
            nc.gpsimd.dma_start(
                out=wo_sb, in_=ap["wo"].rearrange("(a p) m -> p a m", p=128))
        lnw0 = wp.tile([128, 1], F32)
        lnb0 = wp.tile([128, 1], F32)
        b4_sb = wp.tile([128, MO], F32)
        nc.sync.dma_start(out=lnw0, in_=ap["lnw0"])
        nc.sync.dma_start(out=lnb0, in_=ap["lnb0"])
        nc.sync.dma_start(out=b4_sb, in_=ap["b4"])
        ones_sb = wp.tile([128, 1], BF16)
        nc.vector.memset(ones_sb, 1.0 / HID)
        ident = wp.tile([128, 128], BF16)
        make_identity(nc, ident)
        eps_sb = wp.tile([1, 1], F32)
        nc.vector.memset(eps_sb, 1e-5)
        carry = wp.tile([128, 1], F32)

        qk_t = [wp.tile([128, L], BF16, tag=f"qk{i}", name=f"qk{i}") for i in range(4)]
        # vaug[h]: [keys-in-block, jpair, jslot, d+den+pad] fp8; fp8 DoubleRow
        # ldweights needs the stationary dim to be a multiple of 64, so pad
        # the 65 live columns (64 d + ones) to 128 with zeros
        vaug = [wp.tile([128, NKB // 2, 2, 128], F8, tag=f"v{h}", name=f"v{h}")
                for h in range(HPC)]
        for h in range(HPC):
            nc.vector.memset(vaug[h][:, :, :, 65:128], 0.0)
            nc.vector.memset(vaug[h][:, :, :, 64:65], 1.0)

        bounce_in = [dram.tile([HID, CH], F32, name=f"rsin{c}") for c in range(NCH)]
        bounce_out = [dram.tile([HID // TP, CH], F32, name=f"rsout{c}") for c in range(NCH)]

        def pre_phase(c, xc):
            """Stats + transform + soft-prefix scan for chunk c.

            Produces hb (bf16 (x-mu)*rstd tiles, slot 0 = partT) and xq (fp8
            copy).  Emitted one chunk ahead so the stats->transform chain is
            off the PE critical path.
            """
            mu_ps = psst.tile([1, CH], F32, tag="st0", name=f"mu_ps{c}")
            sq_ps = psst.tile([1, CH], F32, tag="st1", name=f"sq_ps{c}")
            for kt in range(8):
                nc.tensor.matmul(mu_ps, ones_sb, xc[:, kt, :],
                                 start=(kt == 0), stop=(kt == 7))
            for kt in range(8):
                sq = tp_.tile([128, CH], BF16, tag="sq", name=f"sq{c}_{kt}")
                nc.vector.tensor_mul(out=sq, in0=xc[:, kt, :], in1=xc[:, kt, :])
                nc.tensor.matmul(sq_ps, ones_sb, sq,
                                 start=(kt == 0), stop=(kt == 7))
            mu_row = rp.tile([1, CH], F32, tag="mu_row", bufs=2)
            nc.vector.tensor_copy(out=mu_row, in_=mu_ps)
            mu_bf = rp.tile([1, CH], BF16, tag="mu_bf", bufs=2)
            nc.vector.tensor_copy(out=mu_bf, in_=mu_ps)
            var_row = rp.tile([1, CH], F32, tag="var", bufs=2)
            nc.vector.scalar_tensor_tensor(out=var_row, in0=mu_row, scalar=-1.0,
                                           in1=mu_ps, op0=OP.mult, op1=OP.mult)
            nc.vector.tensor_add(out=var_row, in0=var_row, in1=sq_ps)
            # rstd = exp(-0.5 * ln(var + eps)): stays in the exp/ln act table
            lnv_row = rp.tile([1, CH], F32, tag="lnv", bufs=2)
            nc.scalar.activation(out=lnv_row, in_=var_row, func=AF.Ln,
                                 bias=eps_sb, scale=1.0)
            rstd_row = rp.tile([1, CH], F32, tag="rstd", bufs=2)
            nc.scalar.activation(out=rstd_row, in_=lnv_row, func=AF.Exp,
                                 scale=-0.5)
            mu_b = bp.tile([128, CH], BF16, tag="mu_b", name=f"mu_b{c}")
            rstd_b = bp.tile([128, CH], F32, tag="rstd_b", name=f"rstd_b{c}")
            nc.gpsimd.partition_broadcast(mu_b, mu_bf)
            nc.gpsimd.partition_broadcast(rstd_b, rstd_row)

            # ---- transformed moving tiles: hb = (x - mu) * rstd (bf16),
            # xq = fp8 copy.  Slot 0 is the soft-prefix-max output.
            hb = hp.tile([128, 8, CH], BF16, tag="hb", name=f"hb{c}")
            xq = hp.tile([128, 8, CH], F8, tag="xq", name=f"xq{c}")
            for kt in range(1, 8):
                nc.vector.tensor_tensor(out=hb[:, kt, :], in0=xc[:, kt, :],
                                        in1=mu_b, op=OP.subtract)
                nc.vector.tensor_mul(out=hb[:, kt, :], in0=hb[:, kt, :],
                                     in1=rstd_b)
                nc.gpsimd.tensor_copy(out=xq[:, kt, :], in_=hb[:, kt, :])

            # ---- soft prefix max on channels 0-127 ----
            h0 = tp_.tile([128, CH], F32, tag="h0", name=f"h0_{c}")
            nc.vector.tensor_tensor(out=h0, in0=xc[:, 0, :], in1=mu_b,
                                    op=OP.subtract)
            nc.vector.tensor_mul(out=h0, in0=h0, in1=rstd_b)
            nc.vector.tensor_scalar(out=h0, in0=h0, scalar1=lnw0, scalar2=lnb0,
                                    op0=OP.mult, op1=OP.add)
            e0 = tp_.tile([128, CH], BF16, tag="e0", name=f"e0_{c}")
            nc.scalar.activation(out=e0, in_=h0, func=AF.Exp, scale=5.0)
            c0 = tp_.tile([128, CH], F32, tag="c0", name=f"c0_{c}")
            nc.vector.tensor_tensor_scan(
                out=c0, data0=e0, data1=e0,
                initial=(0.0 if c == 0 else carry[:, 0:1]),
                op0=OP.add, op1=OP.bypass)
            nc.vector.tensor_copy(out=carry, in_=c0[:, CH - 1:CH])
            nc.scalar.activation(out=hb[:, 0, :], in_=c0, func=AF.Ln)
            nc.gpsimd.tensor_copy(out=xq[:, 0, :], in_=hb[:, 0, :])
            return dict(hb=hb, xq=xq)

        xcs = {0: xc0}
        pres = {0: pre_phase(0, xc0)}
        for c in range(NCH):
            t0, t1 = c * CH, (c + 1) * CH
            xc = xcs[c]
            hb, xq = pres[c]["hb"], pres[c]["xq"]
            orhs = orp.tile([128, KO, CH], BF16, tag="orhs", name=f"orhs{c}")

            # ---- fp8 DoubleRow q/k/v m-tiles: 4 kpairs x 2 n-halves ----
            for mt in range(MF8):
                mm = psmm.tile([128, CH], F32, tag="mm")
                for kp in range(4):
                    for nh in range(2):
                        nc.tensor.matmul(
                            mm[:, nh * 256:(nh + 1) * 256],
                            wq8_sb[:, 2 * kp:2 * kp + 2, mt * 128:(mt + 1) * 128],
                            xq[:, 2 * kp:2 * kp + 2, nh * 256:(nh + 1) * 256],
                            start=(kp == 0 and nh == 0),
                            stop=(kp == 3 and nh == 1), perf_mode=DR)
                if mt < MQK:
                    # descale + rope on this q or k pair-of-heads tile
                    qks = tp_.tile([128, CH], BF16, tag="qks")
                    nc.scalar.activation(out=qks, in_=mm, func=AF.Copy,
                                         scale=1.0 / SW)
                    rot = tp_.tile([128, CH], BF16, tag="rot")
                    nc.vector.tensor_copy(out=rot[0:32], in_=qks[32:64])
                    nc.vector.tensor_copy(out=rot[32:64], in_=qks[0:32])
                    nc.vector.tensor_copy(out=rot[64:96], in_=qks[96:128])
                    nc.vector.tensor_copy(out=rot[96:128], in_=qks[64:96])
                    qc = tp_.tile([128, CH], BF16, tag="qc")
                    nc.vector.tensor_mul(out=qc, in0=qks, in1=cos_sb[:, t0:t1])
                    nc.vector.tensor_mul(out=rot, in0=rot, in1=sin_sb[:, t0:t1])
                    nc.vector.tensor_add(out=qk_t[mt][:, t0:t1], in0=qc, in1=rot)
                else:
                    vi = mt - MQK
                    v8 = tp_.tile([128, CH], BF16, tag="v8")
                    nc.scalar.activation(out=v8, in_=mm, func=AF.Copy,
                                         scale=1.0 / SW)
                    for half in range(2):
                        h = 2 * vi + half
                        for blk in range(CH // KB):
                            jb = (CH // KB) * c + blk
                            tr = pspt.tile([128, 64], BF16, tag="pt")
                            nc.tensor.transpose(
                                tr, v8[64 * half:64 * half + 64,
                                       blk * KB:(blk + 1) * KB],
                                ident[64 * half:64 * half + 64,
                                      64 * half:64 * half + 64])
                            nc.vector.tensor_copy(
                                out=vaug[h][:, jb // 2, jb % 2, 0:64], in_=tr)

            # ---- bf16 p m-tiles + gelu straight from PSUM ----
            for pi in range(MP):
                mm = psmm.tile([128, CH], F32, tag="mm")
                for kt in range(KQ):
                    nc.tensor.matmul(
                        mm, wqb_sb[:, kt, pi * 128:(pi + 1) * 128], hb[:, kt, :],
                        start=(kt == 0), stop=(kt == KQ - 1))
                if sim_safe:
                    sg = tp_.tile([128, CH], BF16, tag="sg")
                    nc.scalar.activation(out=sg, in_=mm, func=AF.Sigmoid,
                                         scale=1.702)
                    nc.vector.tensor_mul(out=orhs[:, 2 + pi, :], in0=mm, in1=sg)
                else:
                    nc.scalar.activation(out=orhs[:, 2 + pi, :], in_=mm,
                                         func=AF.Gelu)

            if c == 0:
                load_wo()

            # ---- attention: heads of a pair share PE row groups 0-63/64-127;
            # AV runs as fp8 DoubleRow over key-block pairs ----
            nblk = (CH // KB) * (c + 1)
            for pair in range(HPC // 2):
                qq = qk_t[2 * pair]
                kk = qk_t[2 * pair + 1]
                ots = [psot.tile([128, CH], F32, tag="ot", name=f"ot{c}_{pair}_{i}")
                       for i in range(2)]
                for jp in range(nblk // 2):
                    dm_e = 2 * jp - (CH // KB) * c
                    qlo_e = KB * dm_e if dm_e >= 0 else 0
                    pe8 = pep.tile([128, 2, 2, CH], F8, tag="pe8",
                                   name=f"pe8_{c}_{pair}_{jp}")
                    for sj in range(2):
                        j = 2 * jp + sj
                        dm = j - (CH // KB) * c
                        qlo = KB * dm if dm >= 0 else 0
                        pts = []
                        for i in range(2):
                            sl = slice(64 * i, 64 * i + 64)
                            pt = pspt.tile([128, CH], F32, tag="pt",
                                           name=f"pt{c}_{pair}_{j}_{i}")
                            nc.tensor.matmul(
                                pt[:, qlo:CH],
                                kk[sl, j * KB:(j + 1) * KB],
                                qq[sl, t0 + qlo:t1],
                                start=True, stop=True)
                            pts.append(pt)
                        if dm >= 0 and qlo > qlo_e:
                            # DoubleRow reads [qlo_e:] of both j slots; the
                            # later block's fully-masked columns must be zero
                            nc.gpsimd.memset(pe8[:, sj, :, qlo_e:qlo], 0.0)
                        for i in range(2):
                            pt = pts[i]
                            if dm >= 0:
                                pst = pep.tile([128, 128], BF16, tag="pst")
                                nc.scalar.activation(out=pst,
                                                     in_=pt[:, qlo:qlo + KB],
                                                     func=AF.Exp)
                                nc.vector.tensor_mul(
                                    out=pe8[:, sj, i, qlo:qlo + KB],
                                    in0=pst, in1=tri_sb)
                                if qlo + KB < CH:
                                    nc.scalar.activation(
                                        out=pe8[:, sj, i, qlo + KB:CH],
                                        in_=pt[:, qlo + KB:CH], func=AF.Exp)
                            else:
                                nc.scalar.activation(out=pe8[:, sj, i, :],
                                                     in_=pt, func=AF.Exp)
                    for i in range(2):
                        h = 2 * pair + i
                        npieces = (CH - qlo_e) // 256
                        for piece in range(npieces):
                            n0 = qlo_e + piece * 256
                            nc.tensor.matmul(
                                ots[i][:, n0:n0 + 256],
                                vaug[h][:, jp, :, :],
                                pe8[:, :, i, n0:n0 + 256],
                                start=(jp == 0 and piece == 0),
                                stop=(jp == nblk // 2 - 1 and
                                      piece == npieces - 1),
                                perf_mode=DR, skip_group_check=True)
                for i in range(2):
                    h = 2 * pair + i
                    ot = ots[i]
                    den = rp.tile([1, CH], BF16, tag="den", bufs=2)
                    nc.vector.reciprocal(out=den, in_=ot[64:65, :])
                    den_b = bp.tile([64, CH], BF16, tag="den_b")
                    nc.gpsimd.partition_broadcast(den_b, den)
                    slot = 0 if h < 2 else 1
                    r0 = 64 * (h % 2)
                    nc.vector.tensor_mul(out=orhs[r0:r0 + 64, slot, :],
                                         in0=ot[0:64, :], in1=den_b)

            # ---- pipeline chunk c+1's x load and pre-phase ----
            if c + 1 < NCH:
                xn = xp.tile([128, 8, CH], BF16, tag="xc", name=f"xc{c + 1}")
                nc.gpsimd.dma_start(out=xn, in_=xt3[:, :, (c + 1) * CH:(c + 2) * CH])
                xcs[c + 1] = xn
                pres[c + 1] = pre_phase(c + 1, xn)

            # ---- out-proj: 8 m-tiles x 10 k-tiles, bf16 ----
            kt_order = list(range(2, KO)) + [0, 1]   # p first, cat last
            bn3 = bounce_in[c][:].rearrange("(a p) t -> p a t", p=128)
            for mg in range(MO // 2):
                st = stp.tile([128, 2 * CH], F32, tag="st")
                for mi in range(2):
                    mt = 2 * mg + mi
                    mm = psmm.tile([128, CH], F32, tag="mm")
                    for ki, kt in enumerate(kt_order):
                        nc.tensor.matmul(
                            mm, wo_sb[:, kt, mt * 128:(mt + 1) * 128],
                            orhs[:, kt, :],
                            start=(ki == 0), stop=(ki == KO - 1))
                    nc.vector.tensor_scalar(
                        out=st[:, mi * CH:(mi + 1) * CH], in0=mm,
                        scalar1=b4_sb[:, mt:mt + 1], scalar2=None,
                        op0=OP.add, op1=OP.bypass)
                nc.gpsimd.dma_start(
                    out=bn3[:, 2 * mg:2 * mg + 2, :],
                    in_=st[:].rearrange("p (a t) -> p a t", a=2))

            # ---- chunked reduce-scatter ----
            if not skip_collective:
                nc.gpsimd.collective_compute(
                    "ReduceScatter", OP.add,
                    ins=[bounce_in[c][:].opt()],
                    outs=[bounce_out[c][:].opt()],
                    replica_groups=RG)
                nc.gpsimd.dma_start(out=out_sh[:, t0:t1],
                                    in_=bounce_out[c][:])

        if debug_partial:
            for c in range(NCH):
                nc.sync.dma_start(out=partial_dbg[:, c * CH:(c + 1) * CH],
                                  in_=bounce_in[c][:])
    nc.compile()
    return nc


# ---------------- host-side sharding ----------------

def _rope_tables():
    inv = 1.0 / (10000.0 ** (np.arange(0, D, 2, dtype=np.float64) / D))
    t = np.arange(L, dtype=np.float64)
    f = t[:, None] * inv[None, :]                 # [L, 32]
    emb = np.concatenate([f, f], axis=1)          # [L, 64]
    cos64 = np.cos(emb).T                         # [64, L]
    sin64 = np.sin(emb).T
    s32 = sin64[0:32]
    sin_signed = np.concatenate([-s32, s32], axis=0)   # [64, L]
    cos2 = np.concatenate([cos64, cos64], axis=0)
    sin2 = np.concatenate([sin_signed, sin_signed], axis=0)
    bf = ml_dtypes.bfloat16
    return cos2.astype(bf), sin2.astype(bf)


def prep_inputs(x, ln_w, ln_b, w_in, w_out, b_out):
    x = np.asarray(x, np.float32)
    ln_w = np.asarray(ln_w, np.float32)
    ln_b = np.asarray(ln_b, np.float32)
    w_in = np.asarray(w_in, np.float32)
    w_out = np.asarray(w_out, np.float32)
    b_out = np.asarray(b_out, np.float32)

    cos2, sin2 = _rope_tables()
    tri = (np.arange(128)[None, :] >= np.arange(128)[:, None]).astype(ml_dtypes.bfloat16)
    lnw0 = ln_w[0:128, None].astype(np.float32)
    lnb0 = ln_b[0:128, None].astype(np.float32)
    b4 = (b_out / TP).reshape(MO, 128).T.astype(np.float32).copy()

    xt_b = [np.ascontiguousarray(x[b].T).astype(ml_dtypes.bfloat16) for b in range(B)]

    in_maps = []
    for core in range(N_CORES):
        b, tpi = divmod(core, TP)
        heads = [HPC * tpi + j for j in range(HPC)]
        # fp8 m-tiles: q0q1, k0k1, q2q3, k2k3, v0v1, v2v3 (64 rows each head)
        rows8 = []
        for pair in range(HPC // 2):
            h0, h1 = heads[2 * pair], heads[2 * pair + 1]
            rows8 += list(range(64 * h0, 64 * h0 + 64))
            rows8 += list(range(64 * h1, 64 * h1 + 64))
            rows8 += list(range(HID + 64 * h0, HID + 64 * h0 + 64))
            rows8 += list(range(HID + 64 * h1, HID + 64 * h1 + 64))
        for h in heads:
            rows8 += list(range(2 * HID + 64 * h, 2 * HID + 64 * h + 64))
        rowsp = list(range(3 * HID + 1024 * tpi, 3 * HID + 1024 * (tpi + 1)))

        c1 = w_in[:, ACC:] @ ln_b[ACC:]
        assert np.abs(c1).max() < 1e-6, (
            "nonzero ln_b[128:] not supported by this build (c1 term dropped)")

        def build_wq(rows, scale):
            W_sh = w_in[np.array(rows), :]
            w_eff = W_sh * ln_w[None, :]
            M = len(rows)
            # k-tile order: kt0 = partT (0.2*W[:, :128]), kt1..7 = ch 128..1023
            wq = np.zeros((KQ * 128, M), np.float32)
            wq[0:128] = 0.2 * W_sh[:, 0:ACC].T * scale
            wq[128:1024] = w_eff[:, ACC:].T * scale
            return wq

        wq8 = build_wq(rows8, SW)
        qs = 1.0 / float(D) ** 0.5                # fold q * D^-0.5 into weights
        wq8[:, 0:128] *= qs
        wq8[:, 256:384] *= qs
        wqb = build_wq(rowsp, 1.0)
        # out-proj shard: columns [256*tpi:...] (o) + [1024+1024*tpi ...] (p)
        ocols = list(range(256 * tpi, 256 * (tpi + 1)))
        pcols = list(range(HID + 1024 * tpi, HID + 1024 * (tpi + 1)))
        wo = w_out[:, ocols + pcols].T                          # [1280, 1024]
        in_maps.append({
            "xt": xt_b[b],
            "wq8": wq8.astype(ml_dtypes.float8_e4m3fn),
            "wqb": wqb.astype(ml_dtypes.bfloat16),
            "wo": np.ascontiguousarray(wo).astype(ml_dtypes.bfloat16),
            "cos2": cos2, "sin2": sin2, "tri": tri,
            "lnw0": lnw0, "lnb0": lnb0, "b4": b4,
        })
    return in_maps


def assemble(results):
    """results: list of 8 per-core dicts with 'out_sh' [256, L] f32."""
    out = np.empty((B, L, HID), np.float32)
    for c in range(N_CORES):
        b, tpi = divmod(c, TP)
        out[b, :, 256 * tpi:256 * (tpi + 1)] = results[c]["out_sh"].T
    return out


_NC_CACHE = {}


def _get_nc():
    if "nc" not in _NC_CACHE:
        _NC_CACHE["nc"] = build_nc()
    return _NC_CACHE["nc"]


def kernel(x, ln_w, ln_b, w_in, w_out, b_out):
    from concourse.bass_utils import run_bass_kernel_spmd
    in_maps = prep_inputs(x, ln_w, ln_b, w_in, w_out, b_out)
    nc = _get_nc()
    res = run_bass_kernel_spmd(nc, in_maps, core_ids=list(range(N_CORES)))
    return assemble(res.results)


# revision 43
# speedup vs baseline: 1.3110x; 1.2763x over previous
"""Trainium2 Bass kernel for nn_Block_56427280335230 (dense transformer block).

Reference semantics (B=2, L=2048, H=16, D=64, HID=1024):
    h = LayerNorm(x) * ln_w + ln_b
    h[..., :128] = cumlogsumexp(h[..., :128] * 5, axis=seq) / 5
    qkvp = h @ w_in.T ; split q,k,v,p
    q,k = rope(q), rope(k)
    o = softmax(q k^T / 8 + causal) v
    out = concat([o, gelu(p)]) @ w_out.T + b_out

Sharding: DP2 x TP4 over 8 NeuronCores (cores 0-3 batch 0, 4-7 batch 1;
heads + qkvp/vp weight columns sharded within each group of 4; chunked
ReduceScatter leaves each core a disjoint 256-channel slice).

Dataflow is feature-major (channels on partitions, tokens free).  The
moving side of the qkvp matmul is (x - mu) * rstd (rstd = exp(-.5 ln(var
+ eps)), computed without sqrt/reciprocal so the activation table never
leaves the exp/ln set except for gelu).  q/k/v projection rows and the
attention AV matmul run as fp8e4 DoubleRow matmuls (2 k-tiles per
instruction at 0.5 cycles/row = 4x bf16 throughput); the p/gelu path and
out-proj stay bf16 since coherent fp8 error there would break the 2e-2
gate.  Weights for the fp8 side are host-scaled by 32 and descaled in the
PSUM eviction.  exp(S^T) writes fp8 directly except diagonal triangle
blocks, which stage through bf16 for the causal mask multiply.
"""
import numpy as np
import ml_dtypes
from contextlib import ExitStack

from concourse import bass, mybir, tile, bacc
from concourse.masks import make_identity

F32 = mybir.dt.float32
BF16 = mybir.dt.bfloat16
F8 = mybir.dt.float8e4
DR = mybir.MatmulPerfMode.DoubleRow

B, L, H, D = 2, 2048, 16, 64
HID = H * D                  # 1024
ACC = HID // 8               # 128 scan channels
N_CORES = 8
TP = 4                       # tensor-parallel group size
HPC = H // TP                # heads per core = 4
CH = 512                     # tokens per chunk
NCH = L // CH                # 4 chunks
KB = 128                     # key block
NKB = L // KB                # 16 key blocks
KQ = 8                       # qkvp contraction tiles (partT + 7 x tiles)
MQK, MV, MP = HPC, HPC // 2, 8
MF8 = MQK + MV               # 6 fp8 m-tiles (q,k,v)
MO = 8                       # out-proj m-tiles (1024 out channels)
KO = 10                      # out-proj contraction tiles (1280 vp shard)
VP_SH = KO * 128             # 1280
SW = 32.0                    # fp8 weight scale
RG = [[0, 1, 2, 3], [4, 5, 6, 7]]

AF = mybir.ActivationFunctionType
OP = mybir.AluOpType


def _fix_act_tables(nc):
    """Retarget exp/ln activation-table loads to the combined
    natural_log_exp set and drop now-redundant loads.

    bass's table chooser picks `exp_and_others` for Exp and `natural_log`
    for Ln, so every Ln<->Exp transition in the rstd/scan chain pays a
    1283 ns table reload on the critical path.  One combined set serves
    both (plus Copy/Identity), leaving only the gelu switches.
    """
    from concourse.hw_specs import get_activation_tables
    tabs = list(get_activation_tables(nc.m.arch).items())
    combined = next(i for i, (n, _) in enumerate(tabs)
                    if "natural_log_exp" in n)
    exp_ln = {i for i, (n, _) in enumerate(tabs)
              if n in ("exp_and_others", "natural_log")}
    for blk in nc.m.functions[0].blocks:
        cur = None
        out = []
        for inst in blk.instructions:
            tn = type(inst).__name__
            if tn == "InstLoadActFuncSet":
                tgt = (combined if inst.act_func_set_id in exp_ln
                       else inst.act_func_set_id)
                if tgt == cur:
                    continue
                inst.act_func_set_id = tgt
                cur = tgt
            elif tn == "InstActivation" and cur is not None:
                assert inst.func in tabs[cur][1], (
                    f"{inst.func} not served by table {tabs[cur][0]}")
            out.append(inst)
        blk.instructions = out


def build_nc(sim_safe=False, debug_partial=False, skip_collective=False,
             pre_pos="before_attn"):
    nc = bacc.Bacc("TRN2", target_bir_lowering=False, debug=False,
                   num_devices=N_CORES)
    ap = {}
    ins_spec = [
        ("xt", [HID, L], BF16),
        ("wq8", [KQ * 128, MF8 * 128], F8),
        ("wqb", [KQ * 128, MP * 128], BF16),
        ("wo", [VP_SH, MO * 128], BF16),
        ("cos2", [128, L], BF16),
        ("sin2", [128, L], BF16),
        ("tri", [128, 128], BF16),
        ("lnw0", [128, 1], F32),
        ("lnb0", [128, 1], F32),
        ("b4", [128, MO], F32),
    ]
    for name, shape, dt in ins_spec:
        ap[name] = nc.dram_tensor(name, shape, dt, kind="ExternalInput").ap()
    out_sh = nc.dram_tensor("out_sh", [HID // TP, L], F32, kind="ExternalOutput").ap()
    if debug_partial:
        partial_dbg = nc.dram_tensor("partial", [HID, L], F32, kind="ExternalOutput").ap()

    with tile.TileContext(nc) as tc, ExitStack() as ctx:
        ctx.enter_context(nc.allow_low_precision(
            reason="bf16/fp8 compute pipeline by design; fp32 accumulation in PSUM"))
        wp = ctx.enter_context(tc.tile_pool(name="wp", bufs=1))
        xp = ctx.enter_context(tc.tile_pool(name="xp", bufs=2))
        hp = ctx.enter_context(tc.tile_pool(name="hp", bufs=2))
        bp = ctx.enter_context(tc.tile_pool(name="bp", bufs=2))
        tp_ = ctx.enter_context(tc.tile_pool(name="tp", bufs=2))
        rp = ctx.enter_context(tc.tile_pool(name="rp", bufs=1))
        pep = ctx.enter_context(tc.tile_pool(name="pep", bufs=4))
        stp = ctx.enter_context(tc.tile_pool(name="stp", bufs=2))
        orp = ctx.enter_context(tc.tile_pool(name="orp", bufs=2))
        psmm = ctx.enter_context(tc.tile_pool(name="psmm", bufs=3, space="PSUM"))
        psst = ctx.enter_context(tc.tile_pool(name="psst", bufs=1, space="PSUM"))
        pspt = ctx.enter_context(tc.tile_pool(name="pspt", bufs=2, space="PSUM"))
        psot = ctx.enter_context(tc.tile_pool(name="psot", bufs=2, space="PSUM"))
        dram = ctx.enter_context(tc.tile_pool(name="dram", bufs=1, space="DRAM"))

        # ---- prefetch x chunk 0 before the weight bulk (HWDGE via the idle
        # SP queue: Pool-sequencer SWDGE descgen costs ~2.8 us per DMA) ----
        xt3 = ap["xt"].rearrange("(a p) t -> p a t", p=128)   # [128, 8, L]
        xc0 = xp.tile([128, 8, CH], BF16, tag="xc", name="xc0")
        nc.sync.dma_start(out=xc0[:, 0:4, :], in_=xt3[:, 0:4, 0:CH])
        nc.sync.dma_start(out=xc0[:, 4:8, :], in_=xt3[:, 4:8, 0:CH])

        # ---- resident weights / constants ----
        wq8_sb = wp.tile([128, KQ, MF8 * 128], F8)
        nc.sync.dma_start(out=wq8_sb,
                          in_=ap["wq8"].rearrange("(a p) m -> p a m", p=128))
        wqb_sb = wp.tile([128, KQ, MP * 128], BF16)
        nc.sync.dma_start(out=wqb_sb,
                          in_=ap["wqb"].rearrange("(a p) m -> p a m", p=128))
        cos_sb = wp.tile([128, L], BF16)
        sin_sb = wp.tile([128, L], BF16)
        tri_sb = wp.tile([128, 128], BF16)
        nc.sync.dma_start(out=cos_sb, in_=ap["cos2"])
        nc.sync.dma_start(out=sin_sb, in_=ap["sin2"])
        nc.sync.dma_start(out=tri_sb, in_=ap["tri"])
        wo_sb = wp.tile([128, KO, MO * 128], BF16)

        def load_wo():
            nc.sync.dma_start(
                out=wo_sb, in_=ap["wo"].rearrange("(a p) m -> p a m", p=128))
        lnw0 = wp.tile([128, 1], F32)
        lnb0 = wp.tile([128, 1], F32)
        b4_sb = wp.tile([128, MO], F32)
        nc.sync.dma_start(out=lnw0, in_=ap["lnw0"])
        nc.sync.dma_start(out=lnb0, in_=ap["lnb0"])
        nc.sync.dma_start(out=b4_sb, in_=ap["b4"])
        ones_sb = wp.tile([128, 1], BF16)
        nc.vector.memset(ones_sb, 1.0 / HID)
        ident = wp.tile([128, 128], BF16)
        make_identity(nc, ident)
        eps_sb = wp.tile([1, 1], F32)
        nc.vector.memset(eps_sb, 1e-5)
        carry = wp.tile([128, 1], F32)

        qk_t = [wp.tile([128, L], BF16, tag=f"qk{i}", name=f"qk{i}") for i in range(4)]
        # vaug[h]: [keys-in-block, jpair, jslot, d+den+pad] fp8; fp8 DoubleRow
        # ldweights needs the stationary dim to be a multiple of 64, so pad
        # the 65 live columns (64 d + ones) to 128 with zeros
        vaug = [wp.tile([128, NKB // 2, 2, 128], F8, tag=f"v{h}", name=f"v{h}")
                for h in range(HPC)]

        bounce_in = [dram.tile([HID, CH], F32, name=f"rsin{c}") for c in range(NCH)]
        bounce_out = [dram.tile([HID // TP, CH], F32, name=f"rsout{c}") for c in range(NCH)]

        def pre_phase(c, xc):
            """Stats + transform + soft-prefix scan for chunk c.

            Produces hb (bf16 (x-mu)*rstd tiles, slot 0 = partT) and xq (fp8
            copy).  Emitted one chunk ahead so the stats->transform chain is
            off the PE critical path.
            """
            # mu on partition row 0, E[x^2] on row 1 of one PSUM bank; the
            # sq group rides the pending-zero region opened by the mu group
            st_ps = psst.tile([65, CH], F32, tag="st0", name=f"st_ps{c}")
            mu_ps = st_ps[0:1, :]
            sq_ps = st_ps[64:65, :]
            for kt in range(8):
                nc.tensor.matmul(mu_ps, ones_sb, xc[:, kt, :],
                                 start=(kt == 0), stop=(kt == 7))
            for kt in range(8):
                sq = tp_.tile([128, CH], BF16, tag="sq", name=f"sq{c}_{kt}")
                nc.vector.tensor_mul(out=sq, in0=xc[:, kt, :], in1=xc[:, kt, :])
                nc.tensor.matmul(sq_ps, ones_sb, sq,
                                 start=(kt == 0), stop=(kt == 7))
            mu_row = rp.tile([1, CH], F32, tag="mu_row", bufs=2)
            nc.vector.tensor_copy(out=mu_row, in_=mu_ps)
            mu_bf = rp.tile([1, CH], BF16, tag="mu_bf", bufs=2)
            nc.vector.tensor_copy(out=mu_bf, in_=mu_ps)
            var_row = rp.tile([1, CH], F32, tag="var", bufs=2)
            nc.vector.scalar_tensor_tensor(out=var_row, in0=mu_row, scalar=-1.0,
                                           in1=mu_ps, op0=OP.mult, op1=OP.mult)
            nc.vector.tensor_add(out=var_row, in0=var_row, in1=sq_ps)
            # rstd = exp(-0.5 * ln(var + eps)): stays in the exp/ln act table
            lnv_row = rp.tile([1, CH], F32, tag="lnv", bufs=2)
            nc.scalar.activation(out=lnv_row, in_=var_row, func=AF.Ln,
                                 bias=eps_sb, scale=1.0)
            rstd_row = rp.tile([1, CH], F32, tag="rstd", bufs=2)
            nc.scalar.activation(out=rstd_row, in_=lnv_row, func=AF.Exp,
                                 scale=-0.5)
            mu_b = bp.tile([128, CH], BF16, tag="mu_b", name=f"mu_b{c}")
            rstd_b = bp.tile([128, CH], F32, tag="rstd_b", name=f"rstd_b{c}")
            nc.gpsimd.partition_broadcast(mu_b, mu_bf)
            nc.gpsimd.partition_broadcast(rstd_b, rstd_row)

            # ---- transformed moving tiles: hb = (x - mu) * rstd (bf16),
            # xq = fp8 copy.  Slot 0 is the soft-prefix-max output.
            hb = hp.tile([128, 8, CH], BF16, tag="hb", name=f"hb{c}")
            xq = hp.tile([128, 8, CH], F8, tag="xq", name=f"xq{c}")
            for kt in range(1, 8):
                nc.vector.tensor_tensor(out=hb[:, kt, :], in0=xc[:, kt, :],
                                        in1=mu_b, op=OP.subtract)
                nc.vector.tensor_mul(out=hb[:, kt, :], in0=hb[:, kt, :],
                                     in1=rstd_b)

            # ---- soft prefix max on channels 0-127 ----
            h0 = tp_.tile([128, CH], F32, tag="h0", name=f"h0_{c}")
            nc.vector.tensor_tensor(out=h0, in0=xc[:, 0, :], in1=mu_b,
                                    op=OP.subtract)
            nc.vector.tensor_mul(out=h0, in0=h0, in1=rstd_b)
            nc.vector.tensor_scalar(out=h0, in0=h0, scalar1=lnw0, scalar2=lnb0,
                                    op0=OP.mult, op1=OP.add)
            e0 = tp_.tile([128, CH], BF16, tag="e0", name=f"e0_{c}")
            nc.scalar.activation(out=e0, in_=h0, func=AF.Exp, scale=5.0)
            c0 = tp_.tile([128, CH], F32, tag="c0", name=f"c0_{c}")
            nc.vector.tensor_tensor_scan(
                out=c0, data0=e0, data1=e0,
                initial=(0.0 if c == 0 else carry[:, 0:1]),
                op0=OP.add, op1=OP.bypass)
            nc.vector.tensor_copy(out=carry, in_=c0[:, CH - 1:CH])
            nc.scalar.activation(out=hb[:, 0, :], in_=c0, func=AF.Ln)

            def emit_xq():
                # fp8 copies deferred past the previous chunk's attention so
                # Pool's in-order queue doesn't stall its den broadcasts
                for kt in range(8):
                    nc.gpsimd.tensor_copy(out=xq[:, kt, :], in_=hb[:, kt, :])
            return dict(hb=hb, xq=xq, emit_xq=emit_xq)

        def make_outproj(c, orhs):
            """Emitters for chunk c's out-proj: 4 m-groups + the RS."""
            kt_order = list(range(2, KO)) + [0, 1]   # p first, cat last
            bn3 = bounce_in[c][:].rearrange("(a p) t -> p a t", p=128)

            def group(mg):
                def emit():
                    st = stp.tile([128, 2 * CH], F32, tag="st")
                    for mi in range(2):
                        mt = 2 * mg + mi
                        mm = psmm.tile([128, CH], F32, tag="mm")
                        for ki, kt in enumerate(kt_order):
                            nc.tensor.matmul(
                                mm, wo_sb[:, kt, mt * 128:(mt + 1) * 128],
                                orhs[:, kt, :],
                                start=(ki == 0), stop=(ki == KO - 1))
                        nc.vector.tensor_scalar(
                            out=st[:, mi * CH:(mi + 1) * CH], in0=mm,
                            scalar1=b4_sb[:, mt:mt + 1], scalar2=None,
                            op0=OP.add, op1=OP.bypass)
                    nc.sync.dma_start(
                        out=bn3[:, 2 * mg:2 * mg + 2, :],
                        in_=st[:].rearrange("p (a t) -> p a t", a=2))
                return emit

            def finish():
                if not skip_collective:
                    nc.gpsimd.collective_compute(
                        "ReduceScatter", OP.add,
                        ins=[bounce_in[c][:].opt()],
                        outs=[bounce_out[c][:].opt()],
                        replica_groups=RG)
                    nc.sync.dma_start(out=out_sh[:, c * CH:(c + 1) * CH],
                                      in_=bounce_out[c][:])
            return [group(mg) for mg in range(MO // 2)] + [finish]

        xcs = {0: xc0}
        pres = {0: pre_phase(0, xc0)}
        pres[0]["emit_xq"]()
        for h in range(HPC):
            nc.gpsimd.memset(vaug[h][:, :, :, 65:128], 0.0)
            nc.gpsimd.memset(vaug[h][:, :, :, 64:65], 1.0)
        og_prev = None
        for c in range(NCH):
            t0, t1 = c * CH, (c + 1) * CH
            xc = xcs[c]
            hb, xq = pres[c]["hb"], pres[c]["xq"]
            orhs = orp.tile([128, KO, CH], BF16, tag="orhs", name=f"orhs{c}")
            if c + 1 < NCH:
                xn = xp.tile([128, 8, CH], BF16, tag="xc", name=f"xc{c + 1}")
                nc.sync.dma_start(out=xn[:, 0:4, :],
                                  in_=xt3[:, 0:4, (c + 1) * CH:(c + 2) * CH])
                nc.sync.dma_start(out=xn[:, 4:8, :],
                                  in_=xt3[:, 4:8, (c + 1) * CH:(c + 2) * CH])
                xcs[c + 1] = xn

            # ---- fp8 DoubleRow q/k/v m-tiles: 4 kpairs x 2 n-halves ----
            for mt in range(MF8):
                mm = psmm.tile([128, CH], F32, tag="mm")
                for kp in range(4):
                    for nh in range(2):
                        nc.tensor.matmul(
                            mm[:, nh * 256:(nh + 1) * 256],
                            wq8_sb[:, 2 * kp:2 * kp + 2, mt * 128:(mt + 1) * 128],
                            xq[:, 2 * kp:2 * kp + 2, nh * 256:(nh + 1) * 256],
                            start=(kp == 0 and nh == 0),
                            stop=(kp == 3 and nh == 1), perf_mode=DR)
                if mt < MQK:
                    # descale + rope on this q or k pair-of-heads tile
                    qks = tp_.tile([128, CH], BF16, tag="qks")
                    nc.scalar.activation(out=qks, in_=mm, func=AF.Copy,
                                         scale=1.0 / SW)
                    rot = tp_.tile([128, CH], BF16, tag="rot")
                    nc.vector.tensor_copy(out=rot[0:32], in_=qks[32:64])
                    nc.vector.tensor_copy(out=rot[32:64], in_=qks[0:32])
                    nc.vector.tensor_copy(out=rot[64:96], in_=qks[96:128])
                    nc.vector.tensor_copy(out=rot[96:128], in_=qks[64:96])
                    qc = tp_.tile([128, CH], BF16, tag="qc")
                    nc.vector.tensor_mul(out=qc, in0=qks, in1=cos_sb[:, t0:t1])
                    nc.vector.tensor_mul(out=rot, in0=rot, in1=sin_sb[:, t0:t1])
                    nc.vector.tensor_add(out=qk_t[mt][:, t0:t1], in0=qc, in1=rot)
                else:
                    vi = mt - MQK
                    v8 = tp_.tile([128, CH], BF16, tag="v8")
                    nc.scalar.activation(out=v8, in_=mm, func=AF.Copy,
                                         scale=1.0 / SW)
                    for half in range(2):
                        h = 2 * vi + half
                        for blk in range(CH // KB):
                            jb = (CH // KB) * c + blk
                            tr = pspt.tile([128, 64], BF16, tag="pt")
                            nc.tensor.transpose(
                                tr, v8[64 * half:64 * half + 64,
                                       blk * KB:(blk + 1) * KB],
                                ident[64 * half:64 * half + 64,
                                      64 * half:64 * half + 64])
                            nc.vector.tensor_copy(
                                out=vaug[h][:, jb // 2, jb % 2, 0:64], in_=tr)

            # ---- bf16 p m-tiles + gelu straight from PSUM ----
            for pi in range(MP):
                mm = psmm.tile([128, CH], F32, tag="mm")
                for kt in range(KQ):
                    nc.tensor.matmul(
                        mm, wqb_sb[:, kt, pi * 128:(pi + 1) * 128], hb[:, kt, :],
                        start=(kt == 0), stop=(kt == KQ - 1))
                if sim_safe:
                    sg = tp_.tile([128, CH], BF16, tag="sg")
                    nc.scalar.activation(out=sg, in_=mm, func=AF.Sigmoid,
                                         scale=1.702)
                    nc.vector.tensor_mul(out=orhs[:, 2 + pi, :], in0=mm, in1=sg)
                else:
                    nc.scalar.activation(out=orhs[:, 2 + pi, :], in_=mm,
                                         func=AF.Gelu)

            if c == 0:
                load_wo()

            # ---- chunk c+1's pre-phase: emitted before attention so its
            # Act ops (Ln/Exp) run ahead of the exp flood and its DVE
            # transform chain overlaps attention ----
            if c + 1 < NCH:
                pres[c + 1] = pre_phase(c + 1, xcs[c + 1])

            # ---- attention: heads of a pair share PE row groups 0-63/64-127;
            # AV runs as fp8 DoubleRow over key-block pairs.  The previous
            # chunk's out-proj m-groups are interleaved between jp groups. ----
            nblk = (CH // KB) * (c + 1)
            og = list(og_prev[0]) if og_prev is not None else []
            og_emitted = 0
            jp_done = 0
            for pair in range(HPC // 2):
                qq = qk_t[2 * pair]
                kk = qk_t[2 * pair + 1]
                ots = [psot.tile([128, CH], F32, tag="ot", name=f"ot{c}_{pair}_{i}")
                       for i in range(2)]
                def emit_av(jp, pe8, qlo_e):
                    for i in range(2):
                        h = 2 * pair + i
                        npieces = (CH - qlo_e) // 256
                        for piece in range(npieces):
                            n0 = qlo_e + piece * 256
                            nc.tensor.matmul(
                                ots[i][:, n0:n0 + 256],
                                vaug[h][:, jp, :, :],
                                pe8[:, :, i, n0:n0 + 256],
                                start=(jp == 0 and piece == 0),
                                stop=(jp == nblk // 2 - 1 and
                                      piece == npieces - 1),
                                perf_mode=DR, skip_group_check=True)

                pending_av = None
                for jp in range(nblk // 2):
                    dm_e = 2 * jp - (CH // KB) * c
                    qlo_e = KB * dm_e if dm_e >= 0 else 0
                    pe8 = pep.tile([128, 2, 2, CH], F8, tag="pe8",
                                   name=f"pe8_{c}_{pair}_{jp}")
                    for sj in range(2):
                        j = 2 * jp + sj
                        dm = j - (CH // KB) * c
                        qlo = KB * dm if dm >= 0 else 0
                        pts = []
                        for i in range(2):
                            sl = slice(64 * i, 64 * i + 64)
                            pt = pspt.tile([128, CH], F32, tag="pt",
                                           name=f"pt{c}_{pair}_{j}_{i}")
                            nc.tensor.matmul(
                                pt[:, qlo:CH],
                                kk[sl, j * KB:(j + 1) * KB],
                                qq[sl, t0 + qlo:t1],
                                start=True, stop=True)
                            pts.append(pt)
                        if dm >= 0 and qlo > qlo_e:
                            # DoubleRow reads [qlo_e:] of both j slots; the
                            # later block's fully-masked columns must be zero
                            nc.vector.memset(pe8[:, sj, :, qlo_e:qlo], 0.0)
                        for i in range(2):
                            pt = pts[i]
                            if dm >= 0:
                                pst = pep.tile([128, 128], BF16, tag="pst")
                                nc.scalar.activation(out=pst,
                                                     in_=pt[:, qlo:qlo + KB],
                                                     func=AF.Exp)
                                nc.vector.tensor_mul(
                                    out=pe8[:, sj, i, qlo:qlo + KB],
                                    in0=pst, in1=tri_sb)
                                if qlo + KB < CH:
                                    nc.scalar.activation(
                                        out=pe8[:, sj, i, qlo + KB:CH],
                                        in_=pt[:, qlo + KB:CH], func=AF.Exp)
                            else:
                                nc.scalar.activation(out=pe8[:, sj, i, :],
                                                     in_=pt, func=AF.Exp)
                    # stagger: AV for jp-1 sits behind jp's PT matmuls in the
                    # PE queue, so PE never blocks on jp's exps directly
                    if pending_av is not None:
                        emit_av(*pending_av)
                    pending_av = (jp, pe8, qlo_e)
                    jp_done += 1
                    while (og_emitted < len(og) and
                           jp_done * (len(og) + 1) >= (og_emitted + 1) * nblk):
                        og[og_emitted]()
                        og_emitted += 1
                emit_av(*pending_av)
                for i in range(2):
                    h = 2 * pair + i
                    ot = ots[i]
                    den = rp.tile([1, CH], BF16, tag="den", bufs=2)
                    nc.vector.reciprocal(out=den, in_=ot[64:65, :])
                    den_b = bp.tile([64, CH], BF16, tag="den_b")
                    nc.gpsimd.partition_broadcast(den_b, den)
                    slot = 0 if h < 2 else 1
                    r0 = 64 * (h % 2)
                    nc.vector.tensor_mul(out=orhs[r0:r0 + 64, slot, :],
                                         in0=ot[0:64, :], in1=den_b)


            while og_emitted < len(og):
                og[og_emitted]()
                og_emitted += 1
            if c + 1 < NCH:
                pres[c + 1]["emit_xq"]()

            # queue this chunk's out-proj; it is emitted interleaved with the
            # NEXT chunk's attention so the exp flood overlaps out-proj PE
            # work and the den/cat chain has a whole chunk of slack
            og_prev = (make_outproj(c, orhs), c)

        # final chunk's out-proj + reduce-scatter
        for emit in og_prev[0]:
            emit()

        if debug_partial:
            for c in range(NCH):
                nc.sync.dma_start(out=partial_dbg[:, c * CH:(c + 1) * CH],
                                  in_=bounce_in[c][:])
    nc.compile()
    _fix_act_tables(nc)
    return nc


# ---------------- host-side sharding ----------------

def _rope_tables():
    inv = 1.0 / (10000.0 ** (np.arange(0, D, 2, dtype=np.float64) / D))
    t = np.arange(L, dtype=np.float64)
    f = t[:, None] * inv[None, :]                 # [L, 32]
    emb = np.concatenate([f, f], axis=1)          # [L, 64]
    cos64 = np.cos(emb).T                         # [64, L]
    sin64 = np.sin(emb).T
    s32 = sin64[0:32]
    sin_signed = np.concatenate([-s32, s32], axis=0)   # [64, L]
    cos2 = np.concatenate([cos64, cos64], axis=0)
    sin2 = np.concatenate([sin_signed, sin_signed], axis=0)
    bf = ml_dtypes.bfloat16
    return cos2.astype(bf), sin2.astype(bf)


def prep_inputs(x, ln_w, ln_b, w_in, w_out, b_out):
    x = np.asarray(x, np.float32)
    ln_w = np.asarray(ln_w, np.float32)
    ln_b = np.asarray(ln_b, np.float32)
    w_in = np.asarray(w_in, np.float32)
    w_out = np.asarray(w_out, np.float32)
    b_out = np.asarray(b_out, np.float32)

    cos2, sin2 = _rope_tables()
    tri = (np.arange(128)[None, :] >= np.arange(128)[:, None]).astype(ml_dtypes.bfloat16)
    lnw0 = ln_w[0:128, None].astype(np.float32)
    lnb0 = ln_b[0:128, None].astype(np.float32)
    b4 = (b_out / TP).reshape(MO, 128).T.astype(np.float32).copy()

    xt_b = [np.ascontiguousarray(x[b].T).astype(ml_dtypes.bfloat16) for b in range(B)]

    in_maps = []
    for core in range(N_CORES):
        b, tpi = divmod(core, TP)
        heads = [HPC * tpi + j for j in range(HPC)]
        # fp8 m-tiles: q0q1, k0k1, q2q3, k2k3, v0v1, v2v3 (64 rows each head)
        rows8 = []
        for pair in range(HPC // 2):
            h0, h1 = heads[2 * pair], heads[2 * pair + 1]
            rows8 += list(range(64 * h0, 64 * h0 + 64))
            rows8 += list(range(64 * h1, 64 * h1 + 64))
            rows8 += list(range(HID + 64 * h0, HID + 64 * h0 + 64))
            rows8 += list(range(HID + 64 * h1, HID + 64 * h1 + 64))
        for h in heads:
            rows8 += list(range(2 * HID + 64 * h, 2 * HID + 64 * h + 64))
        rowsp = list(range(3 * HID + 1024 * tpi, 3 * HID + 1024 * (tpi + 1)))

        c1 = w_in[:, ACC:] @ ln_b[ACC:]
        assert np.abs(c1).max() < 1e-6, (
            "nonzero ln_b[128:] not supported by this build (c1 term dropped)")

        def build_wq(rows, scale):
            W_sh = w_in[np.array(rows), :]
            w_eff = W_sh * ln_w[None, :]
            M = len(rows)
            # k-tile order: kt0 = partT (0.2*W[:, :128]), kt1..7 = ch 128..1023
            wq = np.zeros((KQ * 128, M), np.float32)
            wq[0:128] = 0.2 * W_sh[:, 0:ACC].T * scale
            wq[128:1024] = w_eff[:, ACC:].T * scale
            return wq

        wq8 = build_wq(rows8, SW)
        qs = 1.0 / float(D) ** 0.5                # fold q * D^-0.5 into weights
        wq8[:, 0:128] *= qs
        wq8[:, 256:384] *= qs
        wqb = build_wq(rowsp, 1.0)
        # out-proj shard: columns [256*tpi:...] (o) + [1024+1024*tpi ...] (p)
        ocols = list(range(256 * tpi, 256 * (tpi + 1)))
        pcols = list(range(HID + 1024 * tpi, HID + 1024 * (tpi + 1)))
        wo = w_out[:, ocols + pcols].T                          # [1280, 1024]
        in_maps.append({
            "xt": xt_b[b],
            "wq8": wq8.astype(ml_dtypes.float8_e4m3fn),
            "wqb": wqb.astype(ml_dtypes.bfloat16),
            "wo": np.ascontiguousarray(wo).astype(ml_dtypes.bfloat16),
            "cos2": cos2, "sin2": sin2, "tri": tri,
            "lnw0": lnw0, "lnb0": lnb0, "b4": b4,
        })
    return in_maps


def assemble(results):
    """results: list of 8 per-core dicts with 'out_sh' [256, L] f32."""
    out = np.empty((B, L, HID), np.float32)
    for c in range(N_CORES):
        b, tpi = divmod(c, TP)
        out[b, :, 256 * tpi:256 * (tpi + 1)] = results[c]["out_sh"].T
    return out


_NC_CACHE = {}


def _get_nc():
    if "nc" not in _NC_CACHE:
        _NC_CACHE["nc"] = build_nc()
    return _NC_CACHE["nc"]


def kernel(x, ln_w, ln_b, w_in, w_out, b_out):
    from concourse.bass_utils import run_bass_kernel_spmd
    in_maps = prep_inputs(x, ln_w, ln_b, w_in, w_out, b_out)
    nc = _get_nc()
    res = run_bass_kernel_spmd(nc, in_maps, core_ids=list(range(N_CORES)))
    return assemble(res.results)


# revision 70
# speedup vs baseline: 1.3543x; 1.0330x over previous
"""Trainium2 Bass kernel for nn_Block_56427280335230 (dense transformer block).

Reference semantics (B=2, L=2048, H=16, D=64, HID=1024):
    h = LayerNorm(x) * ln_w + ln_b
    h[..., :128] = cumlogsumexp(h[..., :128] * 5, axis=seq) / 5
    qkvp = h @ w_in.T ; split q,k,v,p
    q,k = rope(q), rope(k)
    o = softmax(q k^T / 8 + causal) v
    out = concat([o, gelu(p)]) @ w_out.T + b_out

Sharding: DP2 x TP4 over 8 NeuronCores (cores 0-3 batch 0, 4-7 batch 1;
heads + qkvp/vp weight columns sharded within each group of 4; chunked
ReduceScatter leaves each core a disjoint 256-channel slice).

Dataflow is feature-major (channels on partitions, tokens free).  The
moving side of the qkvp matmul is (x - mu) * rstd (rstd = exp(-.5 ln(var
+ eps)), computed without sqrt/reciprocal so the activation table never
leaves the exp/ln set except for gelu).  q/k/v projection rows and the
attention AV matmul run as fp8e4 DoubleRow matmuls (2 k-tiles per
instruction at 0.5 cycles/row = 4x bf16 throughput); the p/gelu path and
out-proj stay bf16 since coherent fp8 error there would break the 2e-2
gate.  Weights for the fp8 side are host-scaled by 32 and descaled in the
PSUM eviction.  exp(S^T) writes fp8 directly except diagonal triangle
blocks, which stage through bf16 for the causal mask multiply.
"""
import numpy as np
import ml_dtypes
from contextlib import ExitStack

from concourse import bass, mybir, tile, bacc
from concourse.masks import make_identity

F32 = mybir.dt.float32
BF16 = mybir.dt.bfloat16
F8 = mybir.dt.float8e4
DR = mybir.MatmulPerfMode.DoubleRow

B, L, H, D = 2, 2048, 16, 64
HID = H * D                  # 1024
ACC = HID // 8               # 128 scan channels
N_CORES = 8
TP = 4                       # tensor-parallel group size
HPC = H // TP                # heads per core = 4
CH = 512                     # tokens per chunk
NCH = L // CH                # 4 chunks
KB = 128                     # key block
NKB = L // KB                # 16 key blocks
KQ = 8                       # qkvp contraction tiles (partT + 7 x tiles)
MQK, MV, MP = HPC, HPC // 2, 8
MF8 = MQK + MV               # 6 fp8 m-tiles (q,k,v)
MO = 8                       # out-proj m-tiles (1024 out channels)
KO = 10                      # out-proj contraction tiles (1280 vp shard)
VP_SH = KO * 128             # 1280
SW = 32.0                    # fp8 weight scale
RG = [[0, 1, 2, 3], [4, 5, 6, 7]]

AF = mybir.ActivationFunctionType
OP = mybir.AluOpType


def _fix_act_tables(nc):
    """Retarget exp/ln activation-table loads to the combined
    natural_log_exp set and drop now-redundant loads.

    bass's table chooser picks `exp_and_others` for Exp and `natural_log`
    for Ln, so every Ln<->Exp transition in the rstd/scan chain pays a
    1283 ns table reload on the critical path.  One combined set serves
    both (plus Copy/Identity), leaving only the gelu switches.
    """
    from concourse.hw_specs import get_activation_tables
    tabs = list(get_activation_tables(nc.m.arch).items())
    combined = next(i for i, (n, _) in enumerate(tabs)
                    if "natural_log_exp" in n)
    exp_ln = {i for i, (n, _) in enumerate(tabs)
              if n in ("exp_and_others", "natural_log")}
    for blk in nc.m.functions[0].blocks:
        cur = None
        out = []
        for inst in blk.instructions:
            tn = type(inst).__name__
            if tn == "InstLoadActFuncSet":
                tgt = (combined if inst.act_func_set_id in exp_ln
                       else inst.act_func_set_id)
                if tgt == cur:
                    continue
                inst.act_func_set_id = tgt
                cur = tgt
            elif tn == "InstActivation" and cur is not None:
                assert inst.func in tabs[cur][1], (
                    f"{inst.func} not served by table {tabs[cur][0]}")
            out.append(inst)
        blk.instructions = out


def build_nc(sim_safe=False, debug_partial=False, skip_collective=False,
             pre_pos="before_attn"):
    nc = bacc.Bacc("TRN2", target_bir_lowering=False, debug=False,
                   num_devices=N_CORES)
    ap = {}
    ins_spec = [
        ("xt", [HID, L], BF16),
        ("wq8", [KQ * 128, MF8 * 128], F8),
        ("wqb", [KQ * 128, MP * 128], BF16),
        ("wo", [VP_SH, MO * 128], BF16),
        ("cos2", [128, L], BF16),
        ("sin2", [128, L], BF16),
        ("tri", [128, 128], BF16),
        ("lnw0", [128, 1], F32),
        ("lnb0", [128, 1], F32),
        ("b4", [128, MO], F32),
    ]
    for name, shape, dt in ins_spec:
        ap[name] = nc.dram_tensor(name, shape, dt, kind="ExternalInput").ap()
    out_sh = nc.dram_tensor("out_sh", [HID // TP, L], F32, kind="ExternalOutput").ap()
    if debug_partial:
        partial_dbg = nc.dram_tensor("partial", [HID, L], F32, kind="ExternalOutput").ap()

    with tile.TileContext(nc) as tc, ExitStack() as ctx:
        ctx.enter_context(nc.allow_low_precision(
            reason="bf16/fp8 compute pipeline by design; fp32 accumulation in PSUM"))
        wp = ctx.enter_context(tc.tile_pool(name="wp", bufs=1))
        xp = ctx.enter_context(tc.tile_pool(name="xp", bufs=2))
        hp = ctx.enter_context(tc.tile_pool(name="hp", bufs=2))
        sp8 = ctx.enter_context(tc.tile_pool(name="sp8", bufs=1))
        bp = ctx.enter_context(tc.tile_pool(name="bp", bufs=2))
        tp_ = ctx.enter_context(tc.tile_pool(name="tp", bufs=2))
        rp = ctx.enter_context(tc.tile_pool(name="rp", bufs=1))
        pep = ctx.enter_context(tc.tile_pool(name="pep", bufs=4))
        stp = ctx.enter_context(tc.tile_pool(name="stp", bufs=2))
        orp = ctx.enter_context(tc.tile_pool(name="orp", bufs=2))
        psmm = ctx.enter_context(tc.tile_pool(name="psmm", bufs=3, space="PSUM"))
        psst = ctx.enter_context(tc.tile_pool(name="psst", bufs=1, space="PSUM"))
        pspt = ctx.enter_context(tc.tile_pool(name="pspt", bufs=2, space="PSUM"))
        psot = ctx.enter_context(tc.tile_pool(name="psot", bufs=2, space="PSUM"))
        dram = ctx.enter_context(tc.tile_pool(name="dram", bufs=1, space="DRAM"))

        # ---- prefetch x chunk 0 before the weight bulk (HWDGE via the idle
        # SP queue: Pool-sequencer SWDGE descgen costs ~2.8 us per DMA) ----
        xt3 = ap["xt"].rearrange("(a p) t -> p a t", p=128)   # [128, 8, L]
        xc0 = xp.tile([128, 8, CH], BF16, tag="xc", name="xc0")
        nc.sync.dma_start(out=xc0[:, 0:4, :], in_=xt3[:, 0:4, 0:CH])
        nc.sync.dma_start(out=xc0[:, 4:8, :], in_=xt3[:, 4:8, 0:CH])

        # ---- resident weights / constants ----
        wq8_sb = wp.tile([128, KQ, MF8 * 128], F8)
        nc.sync.dma_start(out=wq8_sb,
                          in_=ap["wq8"].rearrange("(a p) m -> p a m", p=128))
        cos_sb = wp.tile([128, L], BF16)
        sin_sb = wp.tile([128, L], BF16)
        tri_sb = wp.tile([128, 128], BF16)
        nc.sync.dma_start(out=cos_sb, in_=ap["cos2"])
        nc.sync.dma_start(out=sin_sb, in_=ap["sin2"])
        nc.sync.dma_start(out=tri_sb, in_=ap["tri"])
        wqb_sb = wp.tile([128, KQ, MP * 128], BF16)
        wqb3 = ap["wqb"].rearrange("(a p) m -> p a m", p=128)

        def load_wqb():
            # deferred into chunk 0's body so the chunk-1 x prefetch isn't
            # stuck behind these 2 MB at startup; split so p m-tiles 0-3
            # can start after the first half lands
            nc.sync.dma_start(out=wqb_sb[:, :, 0:512], in_=wqb3[:, :, 0:512])
            nc.sync.dma_start(out=wqb_sb[:, :, 512:1024],
                              in_=wqb3[:, :, 512:1024])
        wo_sb = wp.tile([128, KO, MO * 128], BF16)

        def load_wo():
            nc.sync.dma_start(
                out=wo_sb, in_=ap["wo"].rearrange("(a p) m -> p a m", p=128))
        lnw0 = wp.tile([128, 1], F32)
        lnb0 = wp.tile([128, 1], F32)
        b4_sb = wp.tile([128, MO], F32)
        nc.sync.dma_start(out=lnw0, in_=ap["lnw0"])
        nc.sync.dma_start(out=lnb0, in_=ap["lnb0"])
        nc.sync.dma_start(out=b4_sb, in_=ap["b4"])
        ones8 = wp.tile([128, 2, 64], F8)
        nc.vector.memset(ones8, 1.0)
        onesb_sb = wp.tile([128, 64], BF16)
        nc.vector.memset(onesb_sb, 1.0)
        ident = wp.tile([128, 128], BF16)
        make_identity(nc, ident)
        eps_sb = wp.tile([1, 1], F32)
        nc.vector.memset(eps_sb, 1e-5)
        carry = wp.tile([128, 1], F32)

        qk_t = [wp.tile([128, L], BF16, tag=f"qk{i}", name=f"qk{i}") for i in range(4)]
        # vaug[h]: [keys-in-block, jpair, jslot, d+den+pad] fp8; fp8 DoubleRow
        # ldweights needs the stationary dim to be a multiple of 64, so pad
        # the 65 live columns (64 d + ones) to 128 with zeros
        vaug = [wp.tile([128, NKB // 2, 2, 128], F8, tag=f"v{h}", name=f"v{h}")
                for h in range(HPC)]

        bounce_in = [dram.tile([HID, CH], F32, name=f"rsin{c}") for c in range(NCH)]
        bounce_out = [dram.tile([HID // TP, CH], F32, name=f"rsout{c}") for c in range(NCH)]

        def pre_phase(c, xc):
            """Stats + transform + soft-prefix scan for chunk c.

            Produces hb (bf16 (x-mu)*rstd tiles, slot 0 = partT) and xq (fp8
            copy).  Emitted one chunk ahead so the stats->transform chain is
            off the PE critical path.
            """
            # bf16 stats: the pre-phase chain is latency-critical and PE has
            # slack exactly where it runs; fp8-DR stats saved PE cycles but
            # added a serial fp8-conversion pass to the chain
            st_ps = psst.tile([128, CH], F32, tag="st0", name=f"st_ps{c}")
            mu_ps = st_ps[0:1, :]
            sq_ps = st_ps[64:65, :]
            for kt in range(8):
                nc.tensor.matmul(st_ps[0:64, :], onesb_sb, xc[:, kt, :],
                                 start=(kt == 0), stop=(kt == 7))
            # mean broadcast as soon as the mu sums land; the centering subs
            # can then overlap the variance/rstd chain
            mu_row = rp.tile([1, CH], F32, tag="mu_row", bufs=2)
            nc.vector.tensor_scalar(out=mu_row, in0=mu_ps, scalar1=1.0 / HID,
                                    scalar2=None, op0=OP.mult, op1=OP.bypass)
            mu_bf = rp.tile([1, CH], BF16, tag="mu_bf", bufs=2)
            nc.vector.tensor_copy(out=mu_bf, in_=mu_row)
            mu_b = bp.tile([128, CH], BF16, tag="mu_b", name=f"mu_b{c}")
            nc.gpsimd.partition_broadcast(mu_b, mu_bf)

            for kt in range(8):
                sq = sp8.tile([128, CH], BF16, tag="sqb", name=f"sq{c}_{kt}",
                              bufs=2)
                nc.vector.tensor_mul(out=sq, in0=xc[:, kt, :],
                                     in1=xc[:, kt, :])
                nc.tensor.matmul(st_ps[64:128, :], onesb_sb, sq,
                                 start=(kt == 0), stop=(kt == 7))

            # ---- centering subs: only need mu_b ----
            hb = hp.tile([128, 8, CH], BF16, tag="hb", name=f"hb{c}")
            xq = hp.tile([128, 8, CH], F8, tag="xq", name=f"xq{c}")
            for kt in range(1, 8):
                nc.vector.tensor_tensor(out=hb[:, kt - 1, :], in0=xc[:, kt, :],
                                        in1=mu_b, op=OP.subtract)
            h0 = tp_.tile([128, CH], F32, tag="h0", name=f"h0_{c}")
            nc.vector.tensor_tensor(out=h0, in0=xc[:, 0, :], in1=mu_b,
                                    op=OP.subtract)

            var_row = rp.tile([1, CH], F32, tag="var", bufs=2)
            nc.vector.scalar_tensor_tensor(out=var_row, in0=mu_row, scalar=-1.0,
                                           in1=mu_row, op0=OP.mult, op1=OP.mult)
            nc.vector.scalar_tensor_tensor(out=var_row, in0=sq_ps,
                                           scalar=1.0 / HID, in1=var_row,
                                           op0=OP.mult, op1=OP.add)
            # rstd = exp(-0.5 * ln(var + eps)): stays in the exp/ln act table
            lnv_row = rp.tile([1, CH], F32, tag="lnv", bufs=2)
            nc.scalar.activation(out=lnv_row, in_=var_row, func=AF.Ln,
                                 bias=eps_sb, scale=1.0)
            rstd_row = rp.tile([1, CH], F32, tag="rstd", bufs=2)
            nc.scalar.activation(out=rstd_row, in_=lnv_row, func=AF.Exp,
                                 scale=-0.5)
            rstd_b = bp.tile([128, CH], F32, tag="rstd_b", name=f"rstd_b{c}")
            nc.gpsimd.partition_broadcast(rstd_b, rstd_row)

            # ---- rstd scaling pass (slots 0..6 = ch 128..1023; slot 7 =
            # soft-prefix-max, the longest chain, so the first DoubleRow
            # k-pairs need not wait for the scan) ----
            for kt in range(7):
                nc.vector.tensor_mul(out=hb[:, kt, :], in0=hb[:, kt, :],
                                     in1=rstd_b)
            nc.vector.tensor_mul(out=h0, in0=h0, in1=rstd_b)
            nc.vector.tensor_scalar(out=h0, in0=h0, scalar1=lnw0, scalar2=lnb0,
                                    op0=OP.mult, op1=OP.add)
            e0 = tp_.tile([128, CH], BF16, tag="e0", name=f"e0_{c}")
            nc.scalar.activation(out=e0, in_=h0, func=AF.Exp, scale=5.0)
            c0 = tp_.tile([128, CH], F32, tag="c0", name=f"c0_{c}")
            nc.vector.tensor_tensor_scan(
                out=c0, data0=e0, data1=e0,
                initial=(0.0 if c == 0 else carry[:, 0:1]),
                op0=OP.add, op1=OP.bypass)
            nc.vector.tensor_copy(out=carry, in_=c0[:, CH - 1:CH])
            nc.scalar.activation(out=hb[:, 7, :], in_=c0, func=AF.Ln)

            def emit_xq():
                # fp8 copies split across Pool and DVE to halve the serial
                # tail of the pre-phase chain
                for kt in range(8):
                    eng = nc.gpsimd if kt % 2 == 0 else nc.vector
                    eng.tensor_copy(out=xq[:, kt, :], in_=hb[:, kt, :])
            return dict(hb=hb, xq=xq, emit_xq=emit_xq)

        def make_outproj(c, orhs):
            """Emitters for chunk c's out-proj: 4 m-groups + the RS."""
            kt_order = list(range(2, KO)) + [0, 1]   # p first, cat last
            bn3 = bounce_in[c][:].rearrange("(a p) t -> p a t", p=128)

            def group(mg):
                def emit():
                    for mi in range(2):
                        mt = 2 * mg + mi
                        mm = psmm.tile([128, CH], F32, tag="mm")
                        for ki, kt in enumerate(kt_order):
                            nc.tensor.matmul(
                                mm, wo_sb[:, kt, mt * 128:(mt + 1) * 128],
                                orhs[:, kt, :],
                                start=(ki == 0), stop=(ki == KO - 1))
                        st = stp.tile([128, CH], F32, tag="st")
                        nc.vector.tensor_scalar(
                            out=st, in0=mm,
                            scalar1=b4_sb[:, mt:mt + 1], scalar2=None,
                            op0=OP.add, op1=OP.bypass)
                        nc.sync.dma_start(out=bn3[:, mt, :], in_=st)
                return emit

            def finish():
                if not skip_collective:
                    nc.gpsimd.collective_compute(
                        "ReduceScatter", OP.add,
                        ins=[bounce_in[c][:].opt()],
                        outs=[bounce_out[c][:].opt()],
                        replica_groups=RG)
                    nc.sync.dma_start(out=out_sh[:, c * CH:(c + 1) * CH],
                                      in_=bounce_out[c][:])
            return [group(mg) for mg in range(MO // 2)] + [finish]

        xcs = {0: xc0}
        pres = {0: pre_phase(0, xc0)}
        pres[0]["emit_xq"]()
        for h in range(HPC):
            nc.gpsimd.memset(vaug[h][:, :, :, 65:128], 0.0)
            nc.gpsimd.memset(vaug[h][:, :, :, 64:65], 1.0)
        og_prev = None
        for c in range(NCH):
            t0, t1 = c * CH, (c + 1) * CH
            xc = xcs[c]
            hb, xq = pres[c]["hb"], pres[c]["xq"]
            orhs = orp.tile([128, KO, CH], BF16, tag="orhs", name=f"orhs{c}")
            if c + 1 < NCH:
                xn = xp.tile([128, 8, CH], BF16, tag="xc", name=f"xc{c + 1}")
                nc.sync.dma_start(out=xn[:, 0:4, :],
                                  in_=xt3[:, 0:4, (c + 1) * CH:(c + 2) * CH])
                nc.sync.dma_start(out=xn[:, 4:8, :],
                                  in_=xt3[:, 4:8, (c + 1) * CH:(c + 2) * CH])
                xcs[c + 1] = xn
            if c == 0:
                load_wqb()

            # ---- fp8 DoubleRow q/k/v m-tiles: 4 kpairs x 2 n-halves ----
            for mt in range(MF8):
                mm = psmm.tile([128, CH], F32, tag="mm")
                for kp in range(4):
                    for nh in range(2):
                        nc.tensor.matmul(
                            mm[:, nh * 256:(nh + 1) * 256],
                            wq8_sb[:, 2 * kp:2 * kp + 2, mt * 128:(mt + 1) * 128],
                            xq[:, 2 * kp:2 * kp + 2, nh * 256:(nh + 1) * 256],
                            start=(kp == 0 and nh == 0),
                            stop=(kp == 3 and nh == 1), perf_mode=DR)
                if mt < MQK:
                    # descale + rope on this q or k pair-of-heads tile
                    qks = tp_.tile([128, CH], BF16, tag="qks")
                    nc.scalar.activation(out=qks, in_=mm, func=AF.Copy,
                                         scale=1.0 / SW)
                    rot = tp_.tile([128, CH], BF16, tag="rot")
                    nc.vector.tensor_copy(out=rot[0:32], in_=qks[32:64])
                    nc.vector.tensor_copy(out=rot[32:64], in_=qks[0:32])
                    nc.vector.tensor_copy(out=rot[64:96], in_=qks[96:128])
                    nc.vector.tensor_copy(out=rot[96:128], in_=qks[64:96])
                    qc = tp_.tile([128, CH], BF16, tag="qc")
                    nc.vector.tensor_mul(out=qc, in0=qks, in1=cos_sb[:, t0:t1])
                    nc.vector.tensor_mul(out=rot, in0=rot, in1=sin_sb[:, t0:t1])
                    nc.vector.tensor_add(out=qk_t[mt][:, t0:t1], in0=qc, in1=rot)
                else:
                    vi = mt - MQK
                    v8 = tp_.tile([128, CH], BF16, tag="v8")
                    nc.scalar.activation(out=v8, in_=mm, func=AF.Copy,
                                         scale=1.0 / SW)
                    for half in range(2):
                        h = 2 * vi + half
                        for blk in range(CH // KB):
                            jb = (CH // KB) * c + blk
                            tr = pspt.tile([128, 64], BF16, tag="pt")
                            nc.tensor.transpose(
                                tr, v8[64 * half:64 * half + 64,
                                       blk * KB:(blk + 1) * KB],
                                ident[64 * half:64 * half + 64,
                                      64 * half:64 * half + 64])
                            nc.vector.tensor_copy(
                                out=vaug[h][:, jb // 2, jb % 2, 0:64], in_=tr)

            # ---- chunk c+1's pre-phase: emitted before the p m-tiles so the
            # stats/rstd/transform chain overlaps the p matmuls and attention,
            # and its Act ops run ahead of the exp flood.  The xq fp8 copies
            # go right after: den broadcasts behind them in the Pool queue
            # have a whole chunk of slack, xq does not. ----
            if c + 1 < NCH:
                pres[c + 1] = pre_phase(c + 1, xcs[c + 1])
                pres[c + 1]["emit_xq"]()

            # ---- bf16 p m-tiles + gelu straight from PSUM ----
            for pi in range(MP):
                mm = psmm.tile([128, CH], F32, tag="mm")
                for kt in range(KQ):
                    nc.tensor.matmul(
                        mm, wqb_sb[:, kt, pi * 128:(pi + 1) * 128], hb[:, kt, :],
                        start=(kt == 0), stop=(kt == KQ - 1))
                if sim_safe:
                    sg = tp_.tile([128, CH], BF16, tag="sg")
                    nc.scalar.activation(out=sg, in_=mm, func=AF.Sigmoid,
                                         scale=1.702)
                    nc.vector.tensor_mul(out=orhs[:, 2 + pi, :], in0=mm, in1=sg)
                else:
                    nc.scalar.activation(out=orhs[:, 2 + pi, :], in_=mm,
                                         func=AF.Gelu)

            if c == 0:
                load_wo()

            # ---- attention: heads of a pair share PE row groups 0-63/64-127;
            # AV runs as fp8 DoubleRow over key-block pairs.  The previous
            # chunk's out-proj m-groups are interleaved between jp groups. ----
            nblk = (CH // KB) * (c + 1)
            og = list(og_prev[0]) if og_prev is not None else []
            og_emitted = 0
            jp_done = 0
            for pair in range(HPC // 2):
                qq = qk_t[2 * pair]
                kk = qk_t[2 * pair + 1]
                ots = [psot.tile([128, CH], F32, tag="ot", name=f"ot{c}_{pair}_{i}")
                       for i in range(2)]
                def emit_av(jp, pe8, qlo_e):
                    for i in range(2):
                        h = 2 * pair + i
                        npieces = (CH - qlo_e) // 256
                        for piece in range(npieces):
                            n0 = qlo_e + piece * 256
                            nc.tensor.matmul(
                                ots[i][:, n0:n0 + 256],
                                vaug[h][:, jp, :, :],
                                pe8[:, :, i, n0:n0 + 256],
                                start=(jp == 0 and piece == 0),
                                stop=(jp == nblk // 2 - 1 and
                                      piece == npieces - 1),
                                perf_mode=DR, skip_group_check=True)

                pending_av = None
                for jp in range(nblk // 2):
                    dm_e = 2 * jp - (CH // KB) * c
                    qlo_e = KB * dm_e if dm_e >= 0 else 0
                    pe8 = pep.tile([128, 2, 2, CH], F8, tag="pe8",
                                   name=f"pe8_{c}_{pair}_{jp}")
                    for sj in range(2):
                        j = 2 * jp + sj
                        dm = j - (CH // KB) * c
                        qlo = KB * dm if dm >= 0 else 0
                        pts = []
                        for i in range(2):
                            sl = slice(64 * i, 64 * i + 64)
                            pt = pspt.tile([128, CH], F32, tag="pt",
                                           name=f"pt{c}_{pair}_{j}_{i}")
                            nc.tensor.matmul(
                                pt[:, qlo:CH],
                                kk[sl, j * KB:(j + 1) * KB],
                                qq[sl, t0 + qlo:t1],
                                start=True, stop=True)
                            pts.append(pt)
                        if dm >= 0 and qlo > qlo_e:
                            # DoubleRow reads [qlo_e:] of both j slots; the
                            # later block's fully-masked columns must be zero
                            nc.vector.memset(pe8[:, sj, :, qlo_e:qlo], 0.0)
                        for i in range(2):
                            pt = pts[i]
                            if dm >= 0:
                                pst = pep.tile([128, 128], BF16, tag="pst")
                                nc.scalar.activation(out=pst,
                                                     in_=pt[:, qlo:qlo + KB],
                                                     func=AF.Exp)
                                nc.vector.tensor_mul(
                                    out=pe8[:, sj, i, qlo:qlo + KB],
                                    in0=pst, in1=tri_sb)
                                if qlo + KB < CH:
                                    nc.scalar.activation(
                                        out=pe8[:, sj, i, qlo + KB:CH],
                                        in_=pt[:, qlo + KB:CH], func=AF.Exp)
                            else:
                                nc.scalar.activation(out=pe8[:, sj, i, :],
                                                     in_=pt, func=AF.Exp)
                    # stagger: AV for jp-1 sits behind jp's PT matmuls in the
                    # PE queue, so PE never blocks on jp's exps directly
                    if pending_av is not None:
                        emit_av(*pending_av)
                    pending_av = (jp, pe8, qlo_e)
                    jp_done += 1
                    while (og_emitted < len(og) and
                           jp_done * (len(og) + 1) >= (og_emitted + 1) * nblk):
                        og[og_emitted]()
                        og_emitted += 1
                emit_av(*pending_av)
                for i in range(2):
                    h = 2 * pair + i
                    ot = ots[i]
                    den = rp.tile([1, CH], BF16, tag="den", bufs=2)
                    nc.vector.reciprocal(out=den, in_=ot[64:65, :])
                    den_b = bp.tile([64, CH], BF16, tag="den_b")
                    nc.gpsimd.partition_broadcast(den_b, den)
                    slot = 0 if h < 2 else 1
                    r0 = 64 * (h % 2)
                    nc.vector.tensor_mul(out=orhs[r0:r0 + 64, slot, :],
                                         in0=ot[0:64, :], in1=den_b)

            while og_emitted < len(og):
                og[og_emitted]()
                og_emitted += 1

            # queue this chunk's out-proj; it is emitted interleaved with the
            # NEXT chunk's attention so the exp flood overlaps out-proj PE
            # work and the den/cat chain has a whole chunk of slack
            og_prev = (make_outproj(c, orhs), c)

        # final chunk's out-proj + reduce-scatter
        for emit in og_prev[0]:
            emit()

        if debug_partial:
            for c in range(NCH):
                nc.sync.dma_start(out=partial_dbg[:, c * CH:(c + 1) * CH],
                                  in_=bounce_in[c][:])
    nc.compile()
    _fix_act_tables(nc)
    return nc


# ---------------- host-side sharding ----------------

def _rope_tables():
    inv = 1.0 / (10000.0 ** (np.arange(0, D, 2, dtype=np.float64) / D))
    t = np.arange(L, dtype=np.float64)
    f = t[:, None] * inv[None, :]                 # [L, 32]
    emb = np.concatenate([f, f], axis=1)          # [L, 64]
    cos64 = np.cos(emb).T                         # [64, L]
    sin64 = np.sin(emb).T
    s32 = sin64[0:32]
    sin_signed = np.concatenate([-s32, s32], axis=0)   # [64, L]
    cos2 = np.concatenate([cos64, cos64], axis=0)
    sin2 = np.concatenate([sin_signed, sin_signed], axis=0)
    bf = ml_dtypes.bfloat16
    return cos2.astype(bf), sin2.astype(bf)


def prep_inputs(x, ln_w, ln_b, w_in, w_out, b_out):
    x = np.asarray(x, np.float32)
    ln_w = np.asarray(ln_w, np.float32)
    ln_b = np.asarray(ln_b, np.float32)
    w_in = np.asarray(w_in, np.float32)
    w_out = np.asarray(w_out, np.float32)
    b_out = np.asarray(b_out, np.float32)

    cos2, sin2 = _rope_tables()
    tri = (np.arange(128)[None, :] >= np.arange(128)[:, None]).astype(ml_dtypes.bfloat16)
    lnw0 = ln_w[0:128, None].astype(np.float32)
    lnb0 = ln_b[0:128, None].astype(np.float32)
    b4 = (b_out / TP).reshape(MO, 128).T.astype(np.float32).copy()

    xt_b = [np.ascontiguousarray(x[b].T).astype(ml_dtypes.bfloat16) for b in range(B)]

    in_maps = []
    for core in range(N_CORES):
        b, tpi = divmod(core, TP)
        heads = [HPC * tpi + j for j in range(HPC)]
        # fp8 m-tiles: q0q1, k0k1, q2q3, k2k3, v0v1, v2v3 (64 rows each head)
        rows8 = []
        for pair in range(HPC // 2):
            h0, h1 = heads[2 * pair], heads[2 * pair + 1]
            rows8 += list(range(64 * h0, 64 * h0 + 64))
            rows8 += list(range(64 * h1, 64 * h1 + 64))
            rows8 += list(range(HID + 64 * h0, HID + 64 * h0 + 64))
            rows8 += list(range(HID + 64 * h1, HID + 64 * h1 + 64))
        for h in heads:
            rows8 += list(range(2 * HID + 64 * h, 2 * HID + 64 * h + 64))
        rowsp = list(range(3 * HID + 1024 * tpi, 3 * HID + 1024 * (tpi + 1)))

        c1 = w_in[:, ACC:] @ ln_b[ACC:]
        assert np.abs(c1).max() < 1e-6, (
            "nonzero ln_b[128:] not supported by this build (c1 term dropped)")

        def build_wq(rows, scale):
            W_sh = w_in[np.array(rows), :]
            w_eff = W_sh * ln_w[None, :]
            M = len(rows)
            # k-tile order: kt0..6 = ch 128..1023, kt7 = partT (0.2*W[:, :128])
            wq = np.zeros((KQ * 128, M), np.float32)
            wq[0:896] = w_eff[:, ACC:].T * scale
            wq[896:1024] = 0.2 * W_sh[:, 0:ACC].T * scale
            return wq

        wq8 = build_wq(rows8, SW)
        qs = 1.0 / float(D) ** 0.5                # fold q * D^-0.5 into weights
        wq8[:, 0:128] *= qs
        wq8[:, 256:384] *= qs
        wqb = build_wq(rowsp, 1.0)
        # out-proj shard: columns [256*tpi:...] (o) + [1024+1024*tpi ...] (p)
        ocols = list(range(256 * tpi, 256 * (tpi + 1)))
        pcols = list(range(HID + 1024 * tpi, HID + 1024 * (tpi + 1)))
        wo = w_out[:, ocols + pcols].T                          # [1280, 1024]
        in_maps.append({
            "xt": xt_b[b],
            "wq8": wq8.astype(ml_dtypes.float8_e4m3fn),
            "wqb": wqb.astype(ml_dtypes.bfloat16),
            "wo": np.ascontiguousarray(wo).astype(ml_dtypes.bfloat16),
            "cos2": cos2, "sin2": sin2, "tri": tri,
            "lnw0": lnw0, "lnb0": lnb0, "b4": b4,
        })
    return in_maps


def assemble(results):
    """results: list of 8 per-core dicts with 'out_sh' [256, L] f32."""
    out = np.empty((B, L, HID), np.float32)
    for c in range(N_CORES):
        b, tpi = divmod(c, TP)
        out[b, :, 256 * tpi:256 * (tpi + 1)] = results[c]["out_sh"].T
    return out


_NC_CACHE = {}


def _get_nc():
    if "nc" not in _NC_CACHE:
        _NC_CACHE["nc"] = build_nc()
    return _NC_CACHE["nc"]


def kernel(x, ln_w, ln_b, w_in, w_out, b_out):
    from concourse.bass_utils import run_bass_kernel_spmd
    in_maps = prep_inputs(x, ln_w, ln_b, w_in, w_out, b_out)
    nc = _get_nc()
    res = run_bass_kernel_spmd(nc, in_maps, core_ids=list(range(N_CORES)))
    return assemble(res.results)


# revision 77
# speedup vs baseline: 1.3682x; 1.0103x over previous
"""Trainium2 Bass kernel for nn_Block_56427280335230 (dense transformer block).

Reference semantics (B=2, L=2048, H=16, D=64, HID=1024):
    h = LayerNorm(x) * ln_w + ln_b
    h[..., :128] = cumlogsumexp(h[..., :128] * 5, axis=seq) / 5
    qkvp = h @ w_in.T ; split q,k,v,p
    q,k = rope(q), rope(k)
    o = softmax(q k^T / 8 + causal) v
    out = concat([o, gelu(p)]) @ w_out.T + b_out

Sharding: DP2 x TP4 over 8 NeuronCores (cores 0-3 batch 0, 4-7 batch 1;
heads + qkvp/vp weight columns sharded within each group of 4; chunked
ReduceScatter leaves each core a disjoint 256-channel slice).

Dataflow is feature-major (channels on partitions, tokens free).  The
moving side of the qkvp matmul is (x - mu) * rstd (rstd = exp(-.5 ln(var
+ eps)), computed without sqrt/reciprocal so the activation table never
leaves the exp/ln set except for gelu).  q/k/v projection rows and the
attention AV matmul run as fp8e4 DoubleRow matmuls (2 k-tiles per
instruction at 0.5 cycles/row = 4x bf16 throughput); the p/gelu path and
out-proj stay bf16 since coherent fp8 error there would break the 2e-2
gate.  Weights for the fp8 side are host-scaled by 32 and descaled in the
PSUM eviction.  exp(S^T) writes fp8 directly except diagonal triangle
blocks, which stage through bf16 for the causal mask multiply.
"""
import numpy as np
import ml_dtypes
from contextlib import ExitStack

from concourse import bass, mybir, tile, bacc
from concourse.masks import make_identity

F32 = mybir.dt.float32
BF16 = mybir.dt.bfloat16
F8 = mybir.dt.float8e4
DR = mybir.MatmulPerfMode.DoubleRow

B, L, H, D = 2, 2048, 16, 64
HID = H * D                  # 1024
ACC = HID // 8               # 128 scan channels
N_CORES = 8
TP = 4                       # tensor-parallel group size
HPC = H // TP                # heads per core = 4
CH = 512                     # tokens per chunk
NCH = L // CH                # 4 chunks
KB = 128                     # key block
NKB = L // KB                # 16 key blocks
KQ = 8                       # qkvp contraction tiles (partT + 7 x tiles)
MQK, MV, MP = HPC, HPC // 2, 8
MF8 = MQK + MV               # 6 fp8 m-tiles (q,k,v)
MO = 8                       # out-proj m-tiles (1024 out channels)
KO = 10                      # out-proj contraction tiles (1280 vp shard)
VP_SH = KO * 128             # 1280
SW = 32.0                    # fp8 weight scale
RG = [[0, 1, 2, 3], [4, 5, 6, 7]]

AF = mybir.ActivationFunctionType
OP = mybir.AluOpType


def _fix_act_tables(nc):
    """Retarget exp/ln activation-table loads to the combined
    natural_log_exp set and drop now-redundant loads.

    bass's table chooser picks `exp_and_others` for Exp and `natural_log`
    for Ln, so every Ln<->Exp transition in the rstd/scan chain pays a
    1283 ns table reload on the critical path.  One combined set serves
    both (plus Copy/Identity), leaving only the gelu switches.
    """
    from concourse.hw_specs import get_activation_tables
    tabs = list(get_activation_tables(nc.m.arch).items())
    combined = next(i for i, (n, _) in enumerate(tabs)
                    if "natural_log_exp" in n)
    exp_ln = {i for i, (n, _) in enumerate(tabs)
              if n in ("exp_and_others", "natural_log")}
    for blk in nc.m.functions[0].blocks:
        cur = None
        out = []
        for inst in blk.instructions:
            tn = type(inst).__name__
            if tn == "InstLoadActFuncSet":
                tgt = (combined if inst.act_func_set_id in exp_ln
                       else inst.act_func_set_id)
                if tgt == cur:
                    continue
                inst.act_func_set_id = tgt
                cur = tgt
            elif tn == "InstActivation" and cur is not None:
                assert inst.func in tabs[cur][1], (
                    f"{inst.func} not served by table {tabs[cur][0]}")
            out.append(inst)
        blk.instructions = out


def build_nc(sim_safe=False, debug_partial=False, skip_collective=False,
             pre_pos="before_attn"):
    nc = bacc.Bacc("TRN2", target_bir_lowering=False, debug=False,
                   num_devices=N_CORES)
    ap = {}
    ins_spec = [
        ("xt", [HID, L], BF16),
        ("wq8", [KQ * 128, MF8 * 128], F8),
        ("wqb", [KQ * 128, MP * 128], BF16),
        ("wo", [VP_SH, MO * 128], BF16),
        ("cos2", [128, L], BF16),
        ("sin2", [128, L], BF16),
        ("tri", [128, 128], BF16),
        ("lnw0", [128, 1], F32),
        ("lnb0", [128, 1], F32),
        ("b4", [128, MO], F32),
    ]
    for name, shape, dt in ins_spec:
        ap[name] = nc.dram_tensor(name, shape, dt, kind="ExternalInput").ap()
    out_sh = nc.dram_tensor("out_sh", [HID // TP, L], F32, kind="ExternalOutput").ap()
    if debug_partial:
        partial_dbg = nc.dram_tensor("partial", [HID, L], F32, kind="ExternalOutput").ap()

    with tile.TileContext(nc) as tc, ExitStack() as ctx:
        ctx.enter_context(nc.allow_low_precision(
            reason="bf16/fp8 compute pipeline by design; fp32 accumulation in PSUM"))
        wp = ctx.enter_context(tc.tile_pool(name="wp", bufs=1))
        xp = ctx.enter_context(tc.tile_pool(name="xp", bufs=2))
        hp = ctx.enter_context(tc.tile_pool(name="hp", bufs=2))
        sp8 = ctx.enter_context(tc.tile_pool(name="sp8", bufs=1))
        bp = ctx.enter_context(tc.tile_pool(name="bp", bufs=2))
        tp_ = ctx.enter_context(tc.tile_pool(name="tp", bufs=3))
        rp = ctx.enter_context(tc.tile_pool(name="rp", bufs=1))
        pep = ctx.enter_context(tc.tile_pool(name="pep", bufs=5))
        stp = ctx.enter_context(tc.tile_pool(name="stp", bufs=2))
        orp = ctx.enter_context(tc.tile_pool(name="orp", bufs=2))
        psmm = ctx.enter_context(tc.tile_pool(name="psmm", bufs=3, space="PSUM"))
        psst = ctx.enter_context(tc.tile_pool(name="psst", bufs=1, space="PSUM"))
        pspt = ctx.enter_context(tc.tile_pool(name="pspt", bufs=2, space="PSUM"))
        psot = ctx.enter_context(tc.tile_pool(name="psot", bufs=2, space="PSUM"))
        dram = ctx.enter_context(tc.tile_pool(name="dram", bufs=1, space="DRAM"))

        # ---- prefetch x chunk 0 before the weight bulk (HWDGE via the idle
        # SP queue: Pool-sequencer SWDGE descgen costs ~2.8 us per DMA) ----
        xt3 = ap["xt"].rearrange("(a p) t -> p a t", p=128)   # [128, 8, L]
        xc0 = xp.tile([128, 8, CH], BF16, tag="xc", name="xc0")
        for q in range(4):
            nc.sync.dma_start(out=xc0[:, 2 * q:2 * q + 2, :],
                              in_=xt3[:, 2 * q:2 * q + 2, 0:CH])

        # ---- resident weights / constants ----
        wq8_sb = wp.tile([128, KQ, MF8 * 128], F8)
        nc.sync.dma_start(out=wq8_sb,
                          in_=ap["wq8"].rearrange("(a p) m -> p a m", p=128))
        cos_sb = wp.tile([128, L], BF16)
        sin_sb = wp.tile([128, L], BF16)
        tri_sb = wp.tile([128, 128], BF16)
        nc.sync.dma_start(out=cos_sb, in_=ap["cos2"])
        nc.sync.dma_start(out=sin_sb, in_=ap["sin2"])
        nc.sync.dma_start(out=tri_sb, in_=ap["tri"])
        wqb_sb = wp.tile([128, KQ, MP * 128], BF16)
        wqb3 = ap["wqb"].rearrange("(a p) m -> p a m", p=128)

        def load_wqb():
            # deferred into chunk 0's body so the chunk-1 x prefetch isn't
            # stuck behind these 2 MB at startup; split so p m-tiles 0-3
            # can start after the first half lands
            nc.sync.dma_start(out=wqb_sb[:, :, 0:512], in_=wqb3[:, :, 0:512])
            nc.sync.dma_start(out=wqb_sb[:, :, 512:1024],
                              in_=wqb3[:, :, 512:1024])
        wo_sb = wp.tile([128, KO, MO * 128], BF16)

        def load_wo():
            nc.sync.dma_start(
                out=wo_sb, in_=ap["wo"].rearrange("(a p) m -> p a m", p=128))
        lnw0 = wp.tile([128, 1], F32)
        lnb0 = wp.tile([128, 1], F32)
        b4_sb = wp.tile([128, MO], F32)
        nc.sync.dma_start(out=lnw0, in_=ap["lnw0"])
        nc.sync.dma_start(out=lnb0, in_=ap["lnb0"])
        nc.sync.dma_start(out=b4_sb, in_=ap["b4"])
        ones8 = wp.tile([128, 2, 64], F8)
        nc.vector.memset(ones8, 1.0)
        onesb_sb = wp.tile([128, 64], BF16)
        nc.vector.memset(onesb_sb, 1.0)
        ident = wp.tile([128, 128], BF16)
        make_identity(nc, ident)
        eps_sb = wp.tile([1, 1], F32)
        nc.vector.memset(eps_sb, 1e-5)
        carry = wp.tile([128, 1], F32)

        qk_t = [wp.tile([128, L], BF16, tag=f"qk{i}", name=f"qk{i}") for i in range(4)]
        # vaug[h]: [keys-in-block, jpair, jslot, d+den+pad] fp8; fp8 DoubleRow
        # ldweights needs the stationary dim to be a multiple of 64, so pad
        # the 65 live columns (64 d + ones) to 128 with zeros
        vaug = [wp.tile([128, NKB // 2, 2, 128], F8, tag=f"v{h}", name=f"v{h}")
                for h in range(HPC)]

        bounce_in = [dram.tile([HID, CH], F32, name=f"rsin{c}") for c in range(NCH)]
        bounce_out = [dram.tile([HID // TP, CH], F32, name=f"rsout{c}") for c in range(NCH)]

        def pre_phase(c, xc):
            """Stats + transform + soft-prefix scan for chunk c.

            Produces hb (bf16 (x-mu)*rstd tiles, slot 0 = partT) and xq (fp8
            copy).  Emitted one chunk ahead so the stats->transform chain is
            off the PE critical path.
            """
            # bf16 stats: the pre-phase chain is latency-critical and PE has
            # slack exactly where it runs; fp8-DR stats saved PE cycles but
            # added a serial fp8-conversion pass to the chain
            st_ps = psst.tile([128, CH], F32, tag="st0", name=f"st_ps{c}")
            mu_ps = st_ps[0:1, :]
            sq_ps = st_ps[64:65, :]
            for kt in range(8):
                nc.tensor.matmul(st_ps[0:64, :], onesb_sb, xc[:, kt, :],
                                 start=(kt == 0), stop=(kt == 7))
            # mean broadcast as soon as the mu sums land; the centering subs
            # can then overlap the variance/rstd chain
            mu_row = rp.tile([1, CH], F32, tag="mu_row", bufs=2)
            nc.vector.tensor_scalar(out=mu_row, in0=mu_ps, scalar1=1.0 / HID,
                                    scalar2=None, op0=OP.mult, op1=OP.bypass)
            mu_bf = rp.tile([1, CH], BF16, tag="mu_bf", bufs=2)
            nc.vector.tensor_copy(out=mu_bf, in_=mu_row)
            mu_b = bp.tile([128, CH], BF16, tag="mu_b", name=f"mu_b{c}")
            nc.gpsimd.partition_broadcast(mu_b, mu_bf)

            for kt in range(8):
                sq = sp8.tile([128, CH], BF16, tag="sqb", name=f"sq{c}_{kt}",
                              bufs=2)
                nc.vector.tensor_mul(out=sq, in0=xc[:, kt, :],
                                     in1=xc[:, kt, :])
                nc.tensor.matmul(st_ps[64:128, :], onesb_sb, sq,
                                 start=(kt == 0), stop=(kt == 7))

            # ---- centering subs: only need mu_b ----
            hb = hp.tile([128, 8, CH], BF16, tag="hb", name=f"hb{c}")
            xq = hp.tile([128, 8, CH], F8, tag="xq", name=f"xq{c}")
            for kt in range(1, 8):
                nc.vector.tensor_tensor(out=hb[:, kt - 1, :], in0=xc[:, kt, :],
                                        in1=mu_b, op=OP.subtract)
            h0 = tp_.tile([128, CH], F32, tag="h0", name=f"h0_{c}")
            nc.vector.tensor_tensor(out=h0, in0=xc[:, 0, :], in1=mu_b,
                                    op=OP.subtract)

            var_row = rp.tile([1, CH], F32, tag="var", bufs=2)
            nc.vector.scalar_tensor_tensor(out=var_row, in0=mu_row, scalar=-1.0,
                                           in1=mu_row, op0=OP.mult, op1=OP.mult)
            nc.vector.scalar_tensor_tensor(out=var_row, in0=sq_ps,
                                           scalar=1.0 / HID, in1=var_row,
                                           op0=OP.mult, op1=OP.add)
            # rstd = exp(-0.5 * ln(var + eps)): stays in the exp/ln act table
            lnv_row = rp.tile([1, CH], F32, tag="lnv", bufs=2)
            nc.scalar.activation(out=lnv_row, in_=var_row, func=AF.Ln,
                                 bias=eps_sb, scale=1.0)
            rstd_row = rp.tile([1, CH], F32, tag="rstd", bufs=2)
            nc.scalar.activation(out=rstd_row, in_=lnv_row, func=AF.Exp,
                                 scale=-0.5)
            rstd_b = bp.tile([128, CH], F32, tag="rstd_b", name=f"rstd_b{c}")
            nc.gpsimd.partition_broadcast(rstd_b, rstd_row)

            # ---- rstd scaling pass (slots 0..6 = ch 128..1023; slot 7 =
            # soft-prefix-max, the longest chain, so the first DoubleRow
            # k-pairs need not wait for the scan) ----
            for kt in range(7):
                nc.vector.tensor_mul(out=hb[:, kt, :], in0=hb[:, kt, :],
                                     in1=rstd_b)
            nc.vector.tensor_mul(out=h0, in0=h0, in1=rstd_b)
            nc.vector.tensor_scalar(out=h0, in0=h0, scalar1=lnw0, scalar2=lnb0,
                                    op0=OP.mult, op1=OP.add)
            e0 = tp_.tile([128, CH], BF16, tag="e0", name=f"e0_{c}")
            nc.scalar.activation(out=e0, in_=h0, func=AF.Exp, scale=5.0)
            c0 = tp_.tile([128, CH], F32, tag="c0", name=f"c0_{c}")
            nc.vector.tensor_tensor_scan(
                out=c0, data0=e0, data1=e0,
                initial=(0.0 if c == 0 else carry[:, 0:1]),
                op0=OP.add, op1=OP.bypass)
            nc.vector.tensor_copy(out=carry, in_=c0[:, CH - 1:CH])
            nc.scalar.activation(out=hb[:, 7, :], in_=c0, func=AF.Ln)

            def emit_xq():
                # fp8 copies split across Pool and DVE to halve the serial
                # tail of the pre-phase chain
                for kt in range(8):
                    eng = nc.gpsimd if kt % 2 == 0 else nc.vector
                    eng.tensor_copy(out=xq[:, kt, :], in_=hb[:, kt, :])
            return dict(hb=hb, xq=xq, emit_xq=emit_xq)

        def make_outproj(c, orhs):
            """Emitters for chunk c's out-proj: 4 m-groups + the RS."""
            kt_order = list(range(2, KO)) + [0, 1]   # p first, cat last
            bn3 = bounce_in[c][:].rearrange("(a p) t -> p a t", p=128)

            def group(mg):
                def emit():
                    # both m-tiles' p-contractions first, cats after: the cat
                    # inputs (den chain) get ~3.4 us of cover on the final
                    # chunk where nothing else hides them
                    mms = []
                    for mi in range(2):
                        mt = 2 * mg + mi
                        mm = psmm.tile([128, CH], F32, tag="mm")
                        for ki, kt in enumerate(kt_order[:-2]):
                            nc.tensor.matmul(
                                mm, wo_sb[:, kt, mt * 128:(mt + 1) * 128],
                                orhs[:, kt, :],
                                start=(ki == 0), stop=False)
                        mms.append(mm)
                    for mi in range(2):
                        mt = 2 * mg + mi
                        mm = mms[mi]
                        for ki, kt in enumerate(kt_order[-2:]):
                            nc.tensor.matmul(
                                mm, wo_sb[:, kt, mt * 128:(mt + 1) * 128],
                                orhs[:, kt, :],
                                start=False, stop=(ki == 1))
                        st = stp.tile([128, CH], F32, tag="st")
                        nc.vector.tensor_scalar(
                            out=st, in0=mm,
                            scalar1=b4_sb[:, mt:mt + 1], scalar2=None,
                            op0=OP.add, op1=OP.bypass)
                        nc.sync.dma_start(out=bn3[:, mt, :], in_=st)
                return emit

            def finish():
                if not skip_collective:
                    nc.gpsimd.collective_compute(
                        "ReduceScatter", OP.add,
                        ins=[bounce_in[c][:].opt()],
                        outs=[bounce_out[c][:].opt()],
                        replica_groups=RG)
                    nc.sync.dma_start(out=out_sh[:, c * CH:(c + 1) * CH],
                                      in_=bounce_out[c][:])
            return [group(mg) for mg in range(MO // 2)] + [finish]

        xcs = {0: xc0}
        pres = {0: pre_phase(0, xc0)}
        pres[0]["emit_xq"]()
        for h in range(HPC):
            nc.gpsimd.memset(vaug[h][:, :, :, 65:128], 0.0)
            nc.gpsimd.memset(vaug[h][:, :, :, 64:65], 1.0)
        og_prev = None
        for c in range(NCH):
            t0, t1 = c * CH, (c + 1) * CH
            xc = xcs[c]
            hb, xq = pres[c]["hb"], pres[c]["xq"]
            orhs = orp.tile([128, KO, CH], BF16, tag="orhs", name=f"orhs{c}")
            if c + 1 < NCH:
                xn = xp.tile([128, 8, CH], BF16, tag="xc", name=f"xc{c + 1}")
                nc.sync.dma_start(out=xn[:, 0:4, :],
                                  in_=xt3[:, 0:4, (c + 1) * CH:(c + 2) * CH])
                nc.sync.dma_start(out=xn[:, 4:8, :],
                                  in_=xt3[:, 4:8, (c + 1) * CH:(c + 2) * CH])
                xcs[c + 1] = xn
            if c == 0:
                load_wqb()

            # ---- fp8 DoubleRow q/k/v m-tiles: 4 kpairs x 2 n-halves ----
            for mt in range(MF8):
                mm = psmm.tile([128, CH], F32, tag="mm")
                for kp in range(4):
                    for nh in range(2):
                        nc.tensor.matmul(
                            mm[:, nh * 256:(nh + 1) * 256],
                            wq8_sb[:, 2 * kp:2 * kp + 2, mt * 128:(mt + 1) * 128],
                            xq[:, 2 * kp:2 * kp + 2, nh * 256:(nh + 1) * 256],
                            start=(kp == 0 and nh == 0),
                            stop=(kp == 3 and nh == 1), perf_mode=DR)
                if mt < MQK:
                    # descale + rope on this q or k pair-of-heads tile
                    qks = tp_.tile([128, CH], BF16, tag="qks")
                    nc.scalar.activation(out=qks, in_=mm, func=AF.Copy,
                                         scale=1.0 / SW)
                    rot = tp_.tile([128, CH], BF16, tag="rot")
                    nc.vector.tensor_copy(out=rot[0:32], in_=qks[32:64])
                    nc.vector.tensor_copy(out=rot[32:64], in_=qks[0:32])
                    nc.vector.tensor_copy(out=rot[64:96], in_=qks[96:128])
                    nc.vector.tensor_copy(out=rot[96:128], in_=qks[64:96])
                    qc = tp_.tile([128, CH], BF16, tag="qc")
                    nc.vector.tensor_mul(out=qc, in0=qks, in1=cos_sb[:, t0:t1])
                    nc.vector.tensor_mul(out=rot, in0=rot, in1=sin_sb[:, t0:t1])
                    nc.vector.tensor_add(out=qk_t[mt][:, t0:t1], in0=qc, in1=rot)
                else:
                    vi = mt - MQK
                    v8 = tp_.tile([128, CH], BF16, tag="v8", bufs=2)
                    nc.scalar.activation(out=v8, in_=mm, func=AF.Copy,
                                         scale=1.0 / SW)
                    for half in range(2):
                        h = 2 * vi + half
                        for blk in range(CH // KB):
                            jb = (CH // KB) * c + blk
                            tr = pspt.tile([128, 64], BF16, tag="pt")
                            nc.tensor.transpose(
                                tr, v8[64 * half:64 * half + 64,
                                       blk * KB:(blk + 1) * KB],
                                ident[64 * half:64 * half + 64,
                                      64 * half:64 * half + 64])
                            nc.vector.tensor_copy(
                                out=vaug[h][:, jb // 2, jb % 2, 0:64], in_=tr)

            # ---- chunk c+1's pre-phase: emitted before the p m-tiles so the
            # stats/rstd/transform chain overlaps the p matmuls and attention,
            # and its Act ops run ahead of the exp flood.  The xq fp8 copies
            # go right after: den broadcasts behind them in the Pool queue
            # have a whole chunk of slack, xq does not. ----
            if c + 1 < NCH:
                pres[c + 1] = pre_phase(c + 1, xcs[c + 1])
                pres[c + 1]["emit_xq"]()

            # ---- bf16 p m-tiles + gelu straight from PSUM ----
            for pi in range(MP):
                mm = psmm.tile([128, CH], F32, tag="mm")
                for kt in range(KQ):
                    nc.tensor.matmul(
                        mm, wqb_sb[:, kt, pi * 128:(pi + 1) * 128], hb[:, kt, :],
                        start=(kt == 0), stop=(kt == KQ - 1))
                if sim_safe:
                    sg = tp_.tile([128, CH], BF16, tag="sg", bufs=1)
                    nc.scalar.activation(out=sg, in_=mm, func=AF.Sigmoid,
                                         scale=1.702)
                    nc.vector.tensor_mul(out=orhs[:, 2 + pi, :], in0=mm, in1=sg)
                else:
                    nc.scalar.activation(out=orhs[:, 2 + pi, :], in_=mm,
                                         func=AF.Gelu)

            if c == 0:
                load_wo()

            # ---- attention: heads of a pair share PE row groups 0-63/64-127;
            # AV runs as fp8 DoubleRow over key-block pairs.  The previous
            # chunk's out-proj m-groups are interleaved between jp groups. ----
            nblk = (CH // KB) * (c + 1)
            og = list(og_prev[0]) if og_prev is not None else []
            og_emitted = 0
            jp_done = 0
            for pair in range(HPC // 2):
                qq = qk_t[2 * pair]
                kk = qk_t[2 * pair + 1]
                ots = [psot.tile([128, CH], F32, tag="ot", name=f"ot{c}_{pair}_{i}")
                       for i in range(2)]
                def emit_av(jp, pe8, qlo_e):
                    for i in range(2):
                        h = 2 * pair + i
                        npieces = (CH - qlo_e) // 256
                        for piece in range(npieces):
                            n0 = qlo_e + piece * 256
                            nc.tensor.matmul(
                                ots[i][:, n0:n0 + 256],
                                vaug[h][:, jp, :, :],
                                pe8[:, :, i, n0:n0 + 256],
                                start=(jp == 0 and piece == 0),
                                stop=(jp == nblk // 2 - 1 and
                                      piece == npieces - 1),
                                perf_mode=DR, skip_group_check=True)

                pending_av = None
                for jp in range(nblk // 2):
                    dm_e = 2 * jp - (CH // KB) * c
                    qlo_e = KB * dm_e if dm_e >= 0 else 0
                    pe8 = pep.tile([128, 2, 2, CH], F8, tag="pe8",
                                   name=f"pe8_{c}_{pair}_{jp}")
                    for sj in range(2):
                        j = 2 * jp + sj
                        dm = j - (CH // KB) * c
                        qlo = KB * dm if dm >= 0 else 0
                        pts = []
                        for i in range(2):
                            sl = slice(64 * i, 64 * i + 64)
                            pt = pspt.tile([128, CH], F32, tag="pt",
                                           name=f"pt{c}_{pair}_{j}_{i}")
                            nc.tensor.matmul(
                                pt[:, qlo:CH],
                                kk[sl, j * KB:(j + 1) * KB],
                                qq[sl, t0 + qlo:t1],
                                start=True, stop=True)
                            pts.append(pt)
                        if dm >= 0 and qlo > qlo_e:
                            # DoubleRow reads [qlo_e:] of both j slots; the
                            # later block's fully-masked columns must be zero
                            nc.vector.memset(pe8[:, sj, :, qlo_e:qlo], 0.0)
                        for i in range(2):
                            pt = pts[i]
                            if dm >= 0:
                                pst = pep.tile([128, 128], BF16, tag="pst")
                                nc.scalar.activation(out=pst,
                                                     in_=pt[:, qlo:qlo + KB],
                                                     func=AF.Exp)
                                nc.vector.tensor_mul(
                                    out=pe8[:, sj, i, qlo:qlo + KB],
                                    in0=pst, in1=tri_sb)
                                if qlo + KB < CH:
                                    nc.scalar.activation(
                                        out=pe8[:, sj, i, qlo + KB:CH],
                                        in_=pt[:, qlo + KB:CH], func=AF.Exp)
                            else:
                                nc.scalar.activation(out=pe8[:, sj, i, :],
                                                     in_=pt, func=AF.Exp)
                    # stagger: AV for jp-1 sits behind jp's PT matmuls in the
                    # PE queue, so PE never blocks on jp's exps directly
                    if pending_av is not None:
                        emit_av(*pending_av)
                    pending_av = (jp, pe8, qlo_e)
                    jp_done += 1
                    while (og_emitted < len(og) and
                           jp_done * (len(og) - 1) >= (og_emitted + 1) * nblk):
                        og[og_emitted]()
                        og_emitted += 1
                emit_av(*pending_av)
                for i in range(2):
                    h = 2 * pair + i
                    ot = ots[i]
                    den = rp.tile([1, CH], BF16, tag="den", bufs=2)
                    nc.vector.reciprocal(out=den, in_=ot[64:65, :])
                    den_b = bp.tile([64, CH], BF16, tag="den_b")
                    nc.gpsimd.partition_broadcast(den_b, den)
                    slot = 0 if h < 2 else 1
                    r0 = 64 * (h % 2)
                    nc.vector.tensor_mul(out=orhs[r0:r0 + 64, slot, :],
                                         in0=ot[0:64, :], in1=den_b)

            while og_emitted < len(og):
                og[og_emitted]()
                og_emitted += 1

            # queue this chunk's out-proj; it is emitted interleaved with the
            # NEXT chunk's attention so the exp flood overlaps out-proj PE
            # work and the den/cat chain has a whole chunk of slack
            og_prev = (make_outproj(c, orhs), c)

        # final chunk's out-proj + reduce-scatter
        for emit in og_prev[0]:
            emit()

        if debug_partial:
            for c in range(NCH):
                nc.sync.dma_start(out=partial_dbg[:, c * CH:(c + 1) * CH],
                                  in_=bounce_in[c][:])
    nc.compile()
    _fix_act_tables(nc)
    return nc


# ---------------- host-side sharding ----------------

def _rope_tables():
    inv = 1.0 / (10000.0 ** (np.arange(0, D, 2, dtype=np.float64) / D))
    t = np.arange(L, dtype=np.float64)
    f = t[:, None] * inv[None, :]                 # [L, 32]
    emb = np.concatenate([f, f], axis=1)          # [L, 64]
    cos64 = np.cos(emb).T                         # [64, L]
    sin64 = np.sin(emb).T
    s32 = sin64[0:32]
    sin_signed = np.concatenate([-s32, s32], axis=0)   # [64, L]
    cos2 = np.concatenate([cos64, cos64], axis=0)
    sin2 = np.concatenate([sin_signed, sin_signed], axis=0)
    bf = ml_dtypes.bfloat16
    return cos2.astype(bf), sin2.astype(bf)


def prep_inputs(x, ln_w, ln_b, w_in, w_out, b_out):
    x = np.asarray(x, np.float32)
    ln_w = np.asarray(ln_w, np.float32)
    ln_b = np.asarray(ln_b, np.float32)
    w_in = np.asarray(w_in, np.float32)
    w_out = np.asarray(w_out, np.float32)
    b_out = np.asarray(b_out, np.float32)

    cos2, sin2 = _rope_tables()
    tri = (np.arange(128)[None, :] >= np.arange(128)[:, None]).astype(ml_dtypes.bfloat16)
    lnw0 = ln_w[0:128, None].astype(np.float32)
    lnb0 = ln_b[0:128, None].astype(np.float32)
    b4 = (b_out / TP).reshape(MO, 128).T.astype(np.float32).copy()

    xt_b = [np.ascontiguousarray(x[b].T).astype(ml_dtypes.bfloat16) for b in range(B)]

    in_maps = []
    for core in range(N_CORES):
        b, tpi = divmod(core, TP)
        heads = [HPC * tpi + j for j in range(HPC)]
        # fp8 m-tiles: q0q1, k0k1, q2q3, k2k3, v0v1, v2v3 (64 rows each head)
        rows8 = []
        for pair in range(HPC // 2):
            h0, h1 = heads[2 * pair], heads[2 * pair + 1]
            rows8 += list(range(64 * h0, 64 * h0 + 64))
            rows8 += list(range(64 * h1, 64 * h1 + 64))
            rows8 += list(range(HID + 64 * h0, HID + 64 * h0 + 64))
            rows8 += list(range(HID + 64 * h1, HID + 64 * h1 + 64))
        for h in heads:
            rows8 += list(range(2 * HID + 64 * h, 2 * HID + 64 * h + 64))
        rowsp = list(range(3 * HID + 1024 * tpi, 3 * HID + 1024 * (tpi + 1)))

        c1 = w_in[:, ACC:] @ ln_b[ACC:]
        assert np.abs(c1).max() < 1e-6, (
            "nonzero ln_b[128:] not supported by this build (c1 term dropped)")

        def build_wq(rows, scale):
            W_sh = w_in[np.array(rows), :]
            w_eff = W_sh * ln_w[None, :]
            M = len(rows)
            # k-tile order: kt0..6 = ch 128..1023, kt7 = partT (0.2*W[:, :128])
            wq = np.zeros((KQ * 128, M), np.float32)
            wq[0:896] = w_eff[:, ACC:].T * scale
            wq[896:1024] = 0.2 * W_sh[:, 0:ACC].T * scale
            return wq

        wq8 = build_wq(rows8, SW)
        qs = 1.0 / float(D) ** 0.5                # fold q * D^-0.5 into weights
        wq8[:, 0:128] *= qs
        wq8[:, 256:384] *= qs
        wqb = build_wq(rowsp, 1.0)
        # out-proj shard: columns [256*tpi:...] (o) + [1024+1024*tpi ...] (p)
        ocols = list(range(256 * tpi, 256 * (tpi + 1)))
        pcols = list(range(HID + 1024 * tpi, HID + 1024 * (tpi + 1)))
        wo = w_out[:, ocols + pcols].T                          # [1280, 1024]
        in_maps.append({
            "xt": xt_b[b],
            "wq8": wq8.astype(ml_dtypes.float8_e4m3fn),
            "wqb": wqb.astype(ml_dtypes.bfloat16),
            "wo": np.ascontiguousarray(wo).astype(ml_dtypes.bfloat16),
            "cos2": cos2, "sin2": sin2, "tri": tri,
            "lnw0": lnw0, "lnb0": lnb0, "b4": b4,
        })
    return in_maps


def assemble(results):
    """results: list of 8 per-core dicts with 'out_sh' [256, L] f32."""
    out = np.empty((B, L, HID), np.float32)
    for c in range(N_CORES):
        b, tpi = divmod(c, TP)
        out[b, :, 256 * tpi:256 * (tpi + 1)] = results[c]["out_sh"].T
    return out


_NC_CACHE = {}


def _get_nc():
    if "nc" not in _NC_CACHE:
        _NC_CACHE["nc"] = build_nc()
    return _NC_CACHE["nc"]


def kernel(x, ln_w, ln_b, w_in, w_out, b_out):
    from concourse.bass_utils import run_bass_kernel_spmd
    in_maps = prep_inputs(x, ln_w, ln_b, w_in, w_out, b_out)
    nc = _get_nc()
    res = run_bass_kernel_spmd(nc, in_maps, core_ids=list(range(N_CORES)))
    return assemble(res.results)
